# revision 1
# baseline (speedup 1.0000x reference)
"""AdaptiveTokenMixer Trainium2 kernel (8 NeuronCores, pure data parallel).

Per-core algorithm (one batch element per core):
  1. alpha stage: delta_times/valid_mask loaded as per-block rows (one
     contiguous DMA each), sliding windows formed by PE transposes of
     free-dim-shifted slices; masked temporal-decay softmax over K=8 offsets
     blended with host-precomputed (b/(1-b))*softmax(w) (scale-invariant
     under the final renormalization), masked + renormalized -> alpha bf16.
  2. W stage: alpha is written to a DRAM scratch with a SKEWED access
     pattern (single multi-dim DMA), forming banded W^T[m, k] =
     alpha[n0+m, k-m] per 120-position block (m-major 128x128 tiles over a
     zeros-initialized input buffer); loaded back naturally (single DMA) and
     PE-transposed to W[k, m].
  3. Mix: out[m, :] = sum_k W[k, m] * x[n0+k, :] -- one 128x128 @ 128x256
     bf16 matmul per block realizes the K-tap mixing exactly (PSUM f32).
  4. Evict PSUM -> SBUF bf16 (DVE/ACT alternating), single batched DMA out.

Self-contained: hardcodes shapes for B=8, N=4096, d=256, K=8.
"""
import numpy as np
import ml_dtypes

import concourse.bass as bass
import concourse.bacc as bacc
import concourse.mybir as mybir
from concourse import tile
from concourse.bass_utils import run_bass_kernel_spmd

B, N, D, K = 8, 4096, 256, 8
BLK = 120                      # output positions per block
NB = (N + BLK - 1) // BLK      # 35 blocks -> covers 4200 positions
NOUT = NB * BLK                # 4200 rows in padded device output
NPAD = 4224                    # padded input length (>= 34*120 + 136)
KW = 128                       # k-window (contraction) per block
WBLK = KW * KW                 # W scratch elements per block
F = K * NB                     # alpha free size (b-major, p-minor)
BIG = 1024.0

_CACHE = {}


def _build():
    nc = bacc.Bacc("TRN2", target_bir_lowering=False, debug=False,
                   num_devices=B)
    f32 = mybir.dt.float32
    bf16 = mybir.dt.bfloat16

    x_t = nc.dram_tensor("x", [NPAD, D], bf16, kind="ExternalInput")
    dt_t = nc.dram_tensor("dt", [NPAD], f32, kind="ExternalInput")
    vf_t = nc.dram_tensor("vf", [NPAD], f32, kind="ExternalInput")
    bwsm_t = nc.dram_tensor("bwsm", [128, K], f32, kind="ExternalInput")
    idf_t = nc.dram_tensor("idf", [128, 128], f32, kind="ExternalInput")
    idb_t = nc.dram_tensor("idb", [128, 128], bf16, kind="ExternalInput")
    wz_t = nc.dram_tensor("wz", [NB * WBLK], bf16, kind="ExternalInput")
    out_t = nc.dram_tensor("out", [NOUT, D], bf16, kind="ExternalOutput")

    def pb(t):  # [128,(b,p)] view -> [128, b, p] (p innermost, for reduces)
        return bass.AP(t.tensor, t.offset, [t.ap[0], [K, NB], [1, K]])

    def strip(t, p):  # tap-p strip [128, NB] (stride K)
        return bass.AP(t.tensor, t.offset + p, [t.ap[0], [K, NB]])

    def exp_nb(a):  # [128, NB] AP -> [128, NB, (K-rep)]
        return bass.AP(a.tensor, a.offset, [a.ap[0], list(a.ap[1]), [0, K]])

    def exp_k(a):  # [128, K] AP -> [128, (NB-rep), K]
        return bass.AP(a.tensor, a.offset, [a.ap[0], [0, NB], [1, K]])

    with tile.TileContext(nc) as tc:
        with tc.tile_pool(name="alph", bufs=1) as apool, \
             tc.tile_pool(name="mix", bufs=4) as mpool, \
             tc.tile_pool(name="big", bufs=1) as bpool, \
             tc.tile_pool(name="psA", bufs=2, space="PSUM") as psA, \
             tc.tile_pool(name="psB", bufs=3, space="PSUM") as psB:

            # ---- constant / input loads (one DMA each) ----
            ident_f = apool.tile([128, 128], f32)
            nc.sync.dma_start(ident_f[:], idf_t.ap())
            ident_b = apool.tile([128, 128], bf16)
            nc.scalar.dma_start(ident_b[:], idb_t.ap())
            bwsm = apool.tile([128, K], f32)
            nc.sync.dma_start(bwsm[:], bwsm_t.ap())
            dt_rows = apool.tile([35, 136], f32)
            nc.sync.dma_start(dt_rows[:],
                              bass.AP(dt_t, 0, [[BLK, NB], [1, 136]]))
            vf_rows = apool.tile([35, 136], f32)
            nc.sync.dma_start(vf_rows[:],
                              bass.AP(vf_t, 0, [[BLK, NB], [1, 136]]))
            # all 35 x windows in one DMA: x_all[i, b, d] = x[b*120+i, d]
            x_all = bpool.tile([128, NB, D], bf16)
            nc.scalar.dma_start(
                x_all[:], bass.AP(x_t, 0, [[D, 128], [BLK * D, NB], [1, D]]))

            # ---- window strips via PE transpose ----
            dtw = apool.tile([128, F], f32)
            vw = apool.tile([128, F], f32)
            for p in range(K):
                for rows, dst in ((dt_rows, dtw), (vf_rows, vw)):
                    pt = psA.tile([128, NB], f32, tag="win")
                    nc.tensor.transpose(pt[:], rows[:NB, p:p + 128],
                                        ident_f[:NB, :NB])
                    nc.vector.tensor_copy(strip(dst, p), pt[:])

            # ---- alpha stage ----
            t1 = apool.tile([128, F], f32)
            nc.vector.tensor_scalar(t1[:], dtw[:], -1.0, BIG,
                                    mybir.AluOpType.mult, mybir.AluOpType.add)
            cv = apool.tile([128, F], f32)
            nc.vector.tensor_tensor(cv[:], vw[:], exp_nb(strip(vw, 0)),
                                    mybir.AluOpType.mult)
            lg = apool.tile([128, F], f32)
            nc.vector.tensor_tensor(lg[:], t1[:], cv[:], mybir.AluOpType.mult)
            mx = apool.tile([128, NB], f32)
            nc.vector.tensor_reduce(mx[:], pb(lg), mybir.AxisListType.X,
                                    mybir.AluOpType.max)
            ei = apool.tile([128, F], f32)
            nc.vector.tensor_tensor(ei[:], lg[:], exp_nb(mx[:, :]),
                                    mybir.AluOpType.subtract)
            e = apool.tile([128, F], f32)
            nc.scalar.activation(e[:], ei[:], mybir.ActivationFunctionType.Exp)
            s = apool.tile([128, NB], f32)
            nc.vector.tensor_reduce(s[:], pb(e), mybir.AxisListType.X,
                                    mybir.AluOpType.add)
            rcp = apool.tile([128, NB], f32)
            nc.vector.reciprocal(rcp[:], s[:])
            th = apool.tile([128, F], f32)
            nc.vector.tensor_tensor(th[:], e[:], exp_nb(rcp[:, :]),
                                    mybir.AluOpType.mult)
            au = apool.tile([128, F], f32)
            nc.vector.tensor_tensor(au[:], th[:], exp_k(bwsm[:, :]),
                                    mybir.AluOpType.add)
            nc.vector.tensor_tensor(au[:], au[:], cv[:], mybir.AluOpType.mult)
            sa = apool.tile([128, NB], f32)
            nc.vector.tensor_reduce(sa[:], pb(au), mybir.AxisListType.X,
                                    mybir.AluOpType.add)
            nc.vector.tensor_scalar(sa[:], sa[:], 1e-8, None,
                                    mybir.AluOpType.max)
            r = apool.tile([128, NB], f32)
            nc.vector.reciprocal(r[:], sa[:])
            nc.vector.tensor_tensor(r[:], r[:], strip(vw, 0),
                                    mybir.AluOpType.mult)
            af = apool.tile([128, F], bf16)
            nc.vector.tensor_tensor(af[:], au[:], exp_nb(r[:, :]),
                                    mybir.AluOpType.mult)

            # ---- skewed W write (single DMA): W^T[b][m, m+p] = af[m, p, b]
            nc.sync.dma_start(
                bass.AP(wz_t, 0, [[KW + 1, BLK], [WBLK, NB], [1, K]]),
                bass.AP(af.tensor, af.offset, [af.ap[0], [K, NB], [1, K]])[:BLK, :, :])

            # ---- W^T load back (single DMA, natural m-major) ----
            wT_all = bpool.tile([128, NB, KW], bf16)
            nc.scalar.dma_start(
                wT_all[:],
                bass.AP(wz_t, 0, [[KW, 128], [WBLK, NB], [1, KW]]))

            # ---- per-block: PE transpose W^T -> W; banded matmul; evict ----
            out_all = bpool.tile([128, NB, D], bf16)
            for b in range(NB):
                wtp = psA.tile([KW, KW], bf16, tag="wt")
                nc.tensor.transpose(wtp[:], wT_all[:, b, :], ident_b[:])
                wt = mpool.tile([KW, KW], bf16, tag="w")
                if b % 2 == 0:
                    nc.vector.tensor_copy(wt[:], wtp[:])
                else:
                    nc.scalar.copy(wt[:], wtp[:])
                pt = psB.tile([KW, D], f32, tag="mm")
                nc.tensor.matmul(pt[:], wt[:], x_all[:, b, :])
                if b % 2 == 0:
                    nc.scalar.copy(out_all[:BLK, b, :], pt[:BLK, :])
                else:
                    nc.vector.tensor_copy(out_all[:BLK, b, :], pt[:BLK, :])

            # ---- single batched store: out[b*120+i, d] = out_all[i, b, d]
            nc.sync.dma_start(
                bass.AP(out_t, 0, [[D, BLK], [BLK * D, NB], [1, D]]),
                out_all[:BLK, :, :])
    nc.compile()
    return nc


def _get_nc():
    if "nc" not in _CACHE:
        _CACHE["nc"] = _build()
    return _CACHE["nc"]


def _make_in_maps(x, delta_times, valid_mask, w, beta):
    w64 = w.astype(np.float64)
    wsm = np.exp(w64 - w64.max())
    wsm /= wsm.sum()
    b = 1.0 / (1.0 + np.exp(-float(beta[0])))
    bwsm = np.tile((b / (1.0 - b) * wsm)[None, :], (128, 1)).astype(np.float32)
    ident = np.eye(128, dtype=np.float32)
    wz = np.zeros(NB * WBLK, np.float32).astype(ml_dtypes.bfloat16)

    in_maps = []
    for i in range(B):
        xp = np.zeros((NPAD, D), np.float32)
        xp[:N] = x[i]
        dtp = np.zeros(NPAD, np.float32)
        dtp[:N] = delta_times[i]
        vfp = np.zeros(NPAD, np.float32)
        vfp[:N] = valid_mask[i].astype(np.float32)
        in_maps.append({
            "x": xp.astype(ml_dtypes.bfloat16),
            "dt": dtp,
            "vf": vfp,
            "bwsm": bwsm,
            "idf": ident,
            "idb": ident.astype(ml_dtypes.bfloat16),
            "wz": wz,
        })
    return in_maps


def _execute(in_maps, trace=False, **kw):
    nc = _get_nc()
    return run_bass_kernel_spmd(nc, in_maps, core_ids=list(range(B)),
                                trace=trace, **kw)


def kernel(x, delta_times, valid_mask, w, beta):
    in_maps = _make_in_maps(x, delta_times, valid_mask, w, beta)
    kr = _execute(in_maps, trace=False)
    outs = [kr.results[i]["out"][:N].astype(np.float32) for i in range(B)]
    return np.stack(outs, axis=0)



# revision 7
# speedup vs baseline: 1.3943x; 1.3943x over previous
"""AdaptiveTokenMixer Trainium2 kernel (8 NeuronCores, pure data parallel).

Per-core algorithm (one batch element per core), pipelined over 5 groups of
7 position-blocks (BLK=120 outputs per block):
  1. alpha stage: delta_times/valid_mask loaded as per-block rows; ONE PE
     transpose each to [128, NB] column form, tap strips formed by
     partition-shifted copies; masked temporal-decay softmax over K=8 offsets
     blended with host-precomputed (b/(1-b))*softmax(w) -> alpha bf16.
  2. W stage (per group): alpha group-slice written to a DRAM scratch with a
     SKEWED access pattern (banded W^T[m, k] = alpha[n0+m, k-m], m-major
     128x128 tiles over a zeros-initialized buffer); loaded back with an
     XBAR-transposing DMA directly into W[k, m] orientation.
  3. Mix (per block): out[m, :] = sum_k W[k, m] * x[n0+k, :] -- one
     128x128 @ 128x256 bf16 matmul per block (PSUM f32).
  4. Evict PSUM -> SBUF bf16 (vector/gpsimd/scalar rotation), per-group DMA
     store overlapping later groups' matmuls.

Self-contained: hardcodes shapes for B=8, N=4096, d=256, K=8.
"""
import numpy as np
import ml_dtypes

import concourse.bass as bass
import concourse.bacc as bacc
import concourse.mybir as mybir
from concourse import tile
from concourse.bass_utils import run_bass_kernel_spmd

B, N, D, K = 8, 4096, 256, 8
BLK = 120                      # output positions per block
NB = (N + BLK - 1) // BLK      # 35 blocks -> covers 4200 positions
NOUT = NB * BLK                # 4200 rows in padded device output
NPAD = 4224                    # padded input length (>= 34*120 + 136)
KW = 128                       # k-window (contraction) per block
WBLK = KW * KW                 # W scratch elements per block
F = K * NB                     # alpha free size (b-major, p-minor)
BIG = 1024.0
G = 5                          # pipeline groups
GB = NB // G                   # blocks per group (7)

_CACHE = {}


def _build():
    nc = bacc.Bacc("TRN2", target_bir_lowering=False, debug=False,
                   num_devices=B)
    f32 = mybir.dt.float32
    bf16 = mybir.dt.bfloat16

    x_t = nc.dram_tensor("x", [NPAD, D], bf16, kind="ExternalInput")
    dt_t = nc.dram_tensor("dt", [NPAD], f32, kind="ExternalInput")
    vf_t = nc.dram_tensor("vf", [NPAD], f32, kind="ExternalInput")
    bwsm_t = nc.dram_tensor("bwsm", [128, K], f32, kind="ExternalInput")
    idf_t = nc.dram_tensor("idf", [128, 128], f32, kind="ExternalInput")
    wz_t = nc.dram_tensor("wz", [NB * WBLK], bf16, kind="ExternalInput")
    out_t = nc.dram_tensor("out", [NOUT, D], bf16, kind="ExternalOutput")

    def pb(t):  # [128,(b,p)] view -> [128, b, p] (p innermost, for reduces)
        return bass.AP(t.tensor, t.offset, [t.ap[0], [K, NB], [1, K]])

    def strip(t, p):  # tap-p strip [128, NB] (stride K)
        return bass.AP(t.tensor, t.offset + p, [t.ap[0], [K, NB]])

    def exp_nb(a):  # [128, NB] AP -> [128, NB, (K-rep)]
        return bass.AP(a.tensor, a.offset, [a.ap[0], list(a.ap[1]), [0, K]])

    def exp_k(a):  # [128, K] AP -> [128, (NB-rep), K]
        return bass.AP(a.tensor, a.offset, [a.ap[0], [0, NB], [1, K]])

    with tile.TileContext(nc) as tc:
        with tc.tile_pool(name="alph", bufs=1) as apool, \
             tc.tile_pool(name="outg", bufs=3) as opool, \
             tc.tile_pool(name="big", bufs=1) as bpool, \
             tc.tile_pool(name="psA", bufs=2, space="PSUM") as psA, \
             tc.tile_pool(name="psB", bufs=4, space="PSUM") as psB:

            # ---- constant / input loads ----
            ident_f = apool.tile([128, 128], f32)
            nc.sync.dma_start(ident_f[:], idf_t.ap())
            bwsm = apool.tile([128, K], f32)
            nc.sync.dma_start(bwsm[:], bwsm_t.ap())
            dt_rows = apool.tile([35, 136], f32)
            nc.sync.dma_start(dt_rows[:],
                              bass.AP(dt_t, 0, [[BLK, NB], [1, 136]]))
            vf_rows = apool.tile([35, 136], f32)
            nc.sync.dma_start(vf_rows[:],
                              bass.AP(vf_t, 0, [[BLK, NB], [1, 136]]))
            # x windows, loaded per group: x_all[i, b, d] = x[b*120+i, d]
            x_all = bpool.tile([128, NB, D], bf16)
            for g in range(G):
                nc.scalar.dma_start(
                    x_all[:, g * GB:(g + 1) * GB, :],
                    bass.AP(x_t, g * GB * BLK * D,
                            [[D, 128], [BLK * D, GB], [1, D]]))

            # ---- window strips via PE transpose ----
            dtw = apool.tile([128, F], f32)
            vw = apool.tile([128, F], f32)
            for p in range(K):
                for rows, dst in ((dt_rows, dtw), (vf_rows, vw)):
                    pt = psA.tile([128, NB], f32, tag="win")
                    nc.tensor.transpose(pt[:], rows[:NB, p:p + 128],
                                        ident_f[:NB, :NB])
                    nc.vector.tensor_copy(strip(dst, p), pt[:])

            # ---- alpha stage ----
            t1 = apool.tile([128, F], f32)
            nc.vector.tensor_scalar(t1[:], dtw[:], -1.0, BIG,
                                    mybir.AluOpType.mult, mybir.AluOpType.add)
            cv = apool.tile([128, F], f32)
            nc.vector.tensor_tensor(cv[:], vw[:], exp_nb(strip(vw, 0)),
                                    mybir.AluOpType.mult)
            lg = apool.tile([128, F], f32)
            nc.vector.tensor_tensor(lg[:], t1[:], cv[:], mybir.AluOpType.mult)
            mx = apool.tile([128, NB], f32)
            nc.vector.tensor_reduce(mx[:], pb(lg), mybir.AxisListType.X,
                                    mybir.AluOpType.max)
            ei = apool.tile([128, F], f32)
            nc.vector.tensor_tensor(ei[:], lg[:], exp_nb(mx[:, :]),
                                    mybir.AluOpType.subtract)
            e = apool.tile([128, F], f32)
            nc.scalar.activation(e[:], ei[:], mybir.ActivationFunctionType.Exp)
            s = apool.tile([128, NB], f32)
            nc.vector.tensor_reduce(s[:], pb(e), mybir.AxisListType.X,
                                    mybir.AluOpType.add)
            rcp = apool.tile([128, NB], f32)
            nc.vector.reciprocal(rcp[:], s[:])
            th = apool.tile([128, F], f32)
            nc.vector.tensor_tensor(th[:], e[:], exp_nb(rcp[:, :]),
                                    mybir.AluOpType.mult)
            au = apool.tile([128, F], f32)
            nc.vector.tensor_tensor(au[:], th[:], exp_k(bwsm[:, :]),
                                    mybir.AluOpType.add)
            nc.vector.tensor_tensor(au[:], au[:], cv[:], mybir.AluOpType.mult)
            sa = apool.tile([128, NB], f32)
            nc.vector.tensor_reduce(sa[:], pb(au), mybir.AxisListType.X,
                                    mybir.AluOpType.add)
            nc.vector.tensor_scalar(sa[:], sa[:], 1e-8, None,
                                    mybir.AluOpType.max)
            r = apool.tile([128, NB], f32)
            nc.vector.reciprocal(r[:], sa[:])
            nc.vector.tensor_tensor(r[:], r[:], strip(vw, 0),
                                    mybir.AluOpType.mult)
            af = apool.tile([128, F], bf16)
            nc.vector.tensor_tensor(af[:], au[:], exp_nb(r[:, :]),
                                    mybir.AluOpType.mult)

            # ---- skewed W writes (per group): W^T[b][m, m+p] = af[m, p, b]
            for g in range(G):
                nc.sync.dma_start(
                    bass.AP(wz_t, g * GB * WBLK,
                            [[KW + 1, BLK], [WBLK, GB], [1, K]]),
                    bass.AP(af.tensor, af.offset + g * GB * K,
                            [af.ap[0], [K, GB], [1, K]])[:BLK, :, :])

            # ---- XBAR-transposed load back (per group): W[k, b*128+m] ----
            w_all = bpool.tile([128, NB, KW], bf16)
            for g in range(G):
                nc.scalar.dma_start(
                    w_all[:, g * GB:(g + 1) * GB, :],
                    bass.AP(wz_t, g * GB * WBLK, [[KW, GB * KW], [1, KW]]),
                    transpose=True)

            # ---- per-block banded matmul; evict; per-group store ----
            for g in range(G):
                out_g = opool.tile([128, GB, D], bf16, tag="og")
                for j in range(GB):
                    b = g * GB + j
                    pt = psB.tile([KW, D], f32, tag="mm")
                    nc.tensor.matmul(pt[:], w_all[:, b, :], x_all[:, b, :])
                    if b % 2 == 0:
                        nc.vector.tensor_copy(out_g[:BLK, j, :], pt[:BLK, :])
                    else:
                        nc.scalar.copy(out_g[:BLK, j, :], pt[:BLK, :])
                nc.sync.dma_start(
                    bass.AP(out_t, g * GB * BLK * D,
                            [[D, BLK], [BLK * D, GB], [1, D]]),
                    out_g[:BLK, :, :])
    nc.compile()
    return nc


def _get_nc():
    if "nc" not in _CACHE:
        _CACHE["nc"] = _build()
    return _CACHE["nc"]


def _make_in_maps(x, delta_times, valid_mask, w, beta):
    w64 = w.astype(np.float64)
    wsm = np.exp(w64 - w64.max())
    wsm /= wsm.sum()
    b = 1.0 / (1.0 + np.exp(-float(beta[0])))
    bwsm = np.tile((b / (1.0 - b) * wsm)[None, :], (128, 1)).astype(np.float32)
    ident = np.eye(128, dtype=np.float32)
    wz = np.zeros(NB * WBLK, np.float32).astype(ml_dtypes.bfloat16)

    in_maps = []
    for i in range(B):
        xp = np.zeros((NPAD, D), np.float32)
        xp[:N] = x[i]
        dtp = np.zeros(NPAD, np.float32)
        dtp[:N] = delta_times[i]
        vfp = np.zeros(NPAD, np.float32)
        vfp[:N] = valid_mask[i].astype(np.float32)
        in_maps.append({
            "x": xp.astype(ml_dtypes.bfloat16),
            "dt": dtp,
            "vf": vfp,
            "bwsm": bwsm,
            "idf": ident,
            "wz": wz,
        })
    return in_maps


def _execute(in_maps, trace=False, **kw):
    nc = _get_nc()
    return run_bass_kernel_spmd(nc, in_maps, core_ids=list(range(B)),
                                trace=trace, **kw)


def kernel(x, delta_times, valid_mask, w, beta):
    in_maps = _make_in_maps(x, delta_times, valid_mask, w, beta)
    kr = _execute(in_maps, trace=False)
    outs = [kr.results[i]["out"][:N].astype(np.float32) for i in range(B)]
    return np.stack(outs, axis=0)


# revision 13
# speedup vs baseline: 1.4637x; 1.0497x over previous
"""AdaptiveTokenMixer Trainium2 kernel (8 NeuronCores, pure data parallel).

Per-core algorithm (one batch element per core), pipelined over 5 groups of
7 position-blocks (BLK=120 outputs per block):
  1. alpha stage: delta_times/valid_mask loaded as per-block rows; sliding
     windows formed by PE transposes of free-dim-shifted slices (t1 fused
     into the PSUM evictions, split across scalar/vector); masked
     temporal-decay softmax over K=8 offsets (elementwise ops split in half
     across vector/gpsimd) blended with host-precomputed
     (b/(1-b))*softmax(w) -> alpha bf16.
  2. W stage (per group): alpha group-slice written to a per-group DRAM
     scratch with a SKEWED access pattern (banded W^T[m, k] =
     alpha[n0+m, k-m], m-major 128x128 tiles over a zeros-initialized
     buffer); loaded back with an XBAR-transposing DMA directly into
     W[k, m] orientation.
  3. Mix (per block): out[m, :] = sum_k W[k, m] * x[n0+k, :] -- one
     128x128 @ 128x256 bf16 matmul per block (PSUM f32), two blocks per
     PSUM bank.
  4. Evict PSUM -> SBUF bf16 (paired, vector engine), per-group DMA store
     overlapping later groups' matmuls.

Self-contained: hardcodes shapes for B=8, N=4096, d=256, K=8.
"""
import numpy as np
import ml_dtypes

import concourse.bass as bass
import concourse.bacc as bacc
import concourse.mybir as mybir
from concourse import tile
from concourse.bass_utils import run_bass_kernel_spmd

B, N, D, K = 8, 4096, 256, 8
BLK = 120                      # output positions per block
NB = (N + BLK - 1) // BLK      # 35 blocks -> covers 4200 positions
NOUT = NB * BLK                # 4200 rows in padded device output
NPAD = 4224                    # padded input length (>= 34*120 + 136)
KW = 128                       # k-window (contraction) per block
WBLK = KW * KW                 # W scratch elements per block
F = K * NB                     # alpha free size (b-major, p-minor)
BIG = 1024.0
G = 5                          # pipeline groups
GB = NB // G                   # blocks per group (7)
FH = F // 2                    # half free size for split elementwise ops

_CACHE = {}


def _build():
    nc = bacc.Bacc("TRN2", target_bir_lowering=False, debug=False,
                   num_devices=B)
    f32 = mybir.dt.float32
    bf16 = mybir.dt.bfloat16

    x_t = nc.dram_tensor("x", [NPAD, D], bf16, kind="ExternalInput")
    dt_t = nc.dram_tensor("dt", [NPAD], f32, kind="ExternalInput")
    vf_t = nc.dram_tensor("vf", [NPAD], f32, kind="ExternalInput")
    bwsm_t = nc.dram_tensor("bwsm", [128, K], f32, kind="ExternalInput")
    idf_t = nc.dram_tensor("idf", [128, 128], f32, kind="ExternalInput")
    wz_t = [nc.dram_tensor(f"wz{g}", [GB * WBLK], bf16, kind="ExternalInput")
            for g in range(G)]
    out_t = nc.dram_tensor("out", [NOUT, D], bf16, kind="ExternalOutput")

    def pb(t):  # [128,(b,p)] view -> [128, b, p] (p innermost, for reduces)
        return bass.AP(t.tensor, t.offset, [t.ap[0], [K, NB], [1, K]])

    def strip(t, p):  # tap-p strip [128, NB] (stride K)
        return bass.AP(t.tensor, t.offset + p, [t.ap[0], [K, NB]])

    def exp_nb(a):  # [128, NB] AP -> [128, NB, (K-rep)]
        return bass.AP(a.tensor, a.offset, [a.ap[0], list(a.ap[1]), [0, K]])

    def exp_k(a):  # [128, K] AP -> [128, (NB-rep), K]
        return bass.AP(a.tensor, a.offset, [a.ap[0], [0, NB], [1, K]])

    NB1 = 18                  # block split for vector/gpsimd halves
    NB2 = NB - NB1
    F1 = NB1 * K

    def exp_nb_h(a, b0, nb):  # block-range slice of an exp_nb broadcast
        return bass.AP(a.tensor, a.offset + b0 * a.ap[1][0],
                       [a.ap[0], [a.ap[1][0], nb], [0, K]])

    def exp_k_h(a, nb):       # block-range slice of an exp_k broadcast
        return bass.AP(a.tensor, a.offset, [a.ap[0], [0, nb], [1, K]])

    def tt2(out, a, b2, op, b2h=None):
        # tensor_tensor split across vector/gpsimd at block boundary NB1.
        # b2h: optional (half1, half2) pair of pre-sliced in1 APs.
        if b2h is None:
            b2h = (b2[:, :F1], b2[:, F1:])
        nc.vector.tensor_tensor(out[:, :F1], a[:, :F1], b2h[0], op)
        nc.gpsimd.tensor_tensor(out[:, F1:], a[:, F1:], b2h[1], op)

    def pb_h(t, b0, nb):  # block-range [128, b, p] view for split reduces
        return bass.AP(t.tensor, t.offset + b0 * K,
                       [t.ap[0], [K, nb], [1, K]])

    def red2(out, t, op):  # tensor_reduce (free-axis reduce is vector-only)
        nc.vector.tensor_reduce(out[:], pb(t), mybir.AxisListType.X, op)

    with tile.TileContext(nc) as tc:
        with tc.tile_pool(name="alph", bufs=1) as apool, \
             tc.tile_pool(name="outg", bufs=3) as opool, \
             tc.tile_pool(name="big", bufs=1) as bpool, \
             tc.tile_pool(name="psA", bufs=2, space="PSUM") as psA, \
             tc.tile_pool(name="psB", bufs=3, space="PSUM") as psB:

            # ---- input / constant loads (sync queue; dt/vf first) ----
            dt_rows = apool.tile([35, 136], f32)
            nc.sync.dma_start(dt_rows[:],
                              bass.AP(dt_t, 0, [[BLK, NB], [1, 136]]))
            vf_rows = apool.tile([35, 136], f32)
            nc.sync.dma_start(vf_rows[:],
                              bass.AP(vf_t, 0, [[BLK, NB], [1, 136]]))
            ident_f = apool.tile([128, 128], f32)
            nc.sync.dma_start(ident_f[:], idf_t.ap())
            bwsm = apool.tile([128, K], f32)
            nc.sync.dma_start(bwsm[:], bwsm_t.ap())
            # x windows, loaded per group: x_all[i, b, d] = x[b*120+i, d]
            x_all = bpool.tile([128, NB, D], bf16)
            for g in range(G):
                nc.sync.dma_start(
                    x_all[:, g * GB:(g + 1) * GB, :],
                    bass.AP(x_t, g * GB * BLK * D,
                            [[D, 128], [BLK * D, GB], [1, D]]))

            # ---- window strips via PE transpose ----
            # t1 = BIG - dt window is fused into the dt-strip evictions.
            t1 = apool.tile([128, F], f32)
            vw = apool.tile([128, F], f32)
            for p in range(K):
                ptd = psA.tile([128, NB], f32, tag="win")
                nc.tensor.transpose(ptd[:], dt_rows[:NB, p:p + 128],
                                    ident_f[:NB, :NB])
                ptv = psA.tile([128, NB], f32, tag="win")
                nc.tensor.transpose(ptv[:], vf_rows[:NB, p:p + 128],
                                    ident_f[:NB, :NB])
                nc.scalar.activation(strip(t1, p), ptd[:],
                                     mybir.ActivationFunctionType.Copy,
                                     bias=BIG, scale=-1.0)
                nc.vector.tensor_copy(strip(vw, p), ptv[:])

            # ---- alpha stage ----
            vw0 = strip(vw, 0)
            cv = apool.tile([128, F], f32)
            tt2(cv, vw, None, mybir.AluOpType.mult,
                b2h=(exp_nb_h(vw0, 0, NB1), exp_nb_h(vw0, NB1, NB2)))
            lg = apool.tile([128, F], f32)
            tt2(lg, t1, cv, mybir.AluOpType.mult)
            mx = apool.tile([128, NB], f32)
            red2(mx, lg, mybir.AluOpType.max)
            ei = apool.tile([128, F], f32)
            tt2(ei, lg, None, mybir.AluOpType.subtract,
                b2h=(exp_nb_h(mx[:, :], 0, NB1), exp_nb_h(mx[:, :], NB1, NB2)))
            e = apool.tile([128, F], f32)
            nc.scalar.activation(e[:], ei[:], mybir.ActivationFunctionType.Exp)
            s = apool.tile([128, NB], f32)
            red2(s, e, mybir.AluOpType.add)
            rcp = apool.tile([128, NB], f32)
            nc.vector.reciprocal(rcp[:], s[:])
            th = apool.tile([128, F], f32)
            tt2(th, e, None, mybir.AluOpType.mult,
                b2h=(exp_nb_h(rcp[:, :], 0, NB1),
                     exp_nb_h(rcp[:, :], NB1, NB2)))
            au = apool.tile([128, F], f32)
            tt2(au, th, None, mybir.AluOpType.add,
                b2h=(exp_k_h(bwsm[:, :], NB1), exp_k_h(bwsm[:, :], NB2)))
            tt2(au, au, cv, mybir.AluOpType.mult)
            sa = apool.tile([128, NB], f32)
            red2(sa, au, mybir.AluOpType.add)
            nc.vector.tensor_scalar(sa[:], sa[:], 1e-8, None,
                                    mybir.AluOpType.max)
            r = apool.tile([128, NB], f32)
            nc.vector.reciprocal(r[:], sa[:])
            nc.vector.tensor_tensor(r[:], r[:], strip(vw, 0),
                                    mybir.AluOpType.mult)
            af = apool.tile([128, F], bf16)
            tt2(af, au, None, mybir.AluOpType.mult,
                b2h=(exp_nb_h(r[:, :], 0, NB1), exp_nb_h(r[:, :], NB1, NB2)))

            # ---- skewed W writes (per group): W^T[b][m, m+p] = af[m, p, b]
            for g in range(G):
                nc.sync.dma_start(
                    bass.AP(wz_t[g], 0, [[KW + 1, BLK], [WBLK, GB], [1, K]]),
                    bass.AP(af.tensor, af.offset + g * GB * K,
                            [af.ap[0], [K, GB], [1, K]])[:BLK, :, :])

            # ---- XBAR-transposed load back (per group): W[k, b*128+m] ----
            w_all = bpool.tile([128, NB, KW], bf16)
            for g in range(G):
                nc.scalar.dma_start(
                    w_all[:, g * GB:(g + 1) * GB, :],
                    bass.AP(wz_t[g], 0, [[KW, GB * KW], [1, KW]]),
                    transpose=True)

            # ---- per-block banded matmul; paired evict; per-group store ----
            for g in range(G):
                out_g = opool.tile([128, GB, D], bf16, tag="og")
                for j in range(0, GB, 2):
                    b = g * GB + j
                    npair = min(2, GB - j)
                    pt = psB.tile([KW, 2 * D], f32, tag="mm")
                    for q in range(npair):
                        nc.tensor.matmul(pt[:, q * D:(q + 1) * D],
                                         w_all[:, b + q, :],
                                         x_all[:, b + q, :])
                    nc.vector.tensor_copy(
                        out_g[:BLK, j:j + npair, :],
                        pt[:BLK, :npair * D])
                nc.sync.dma_start(
                    bass.AP(out_t, g * GB * BLK * D,
                            [[D, BLK], [BLK * D, GB], [1, D]]),
                    out_g[:BLK, :, :])
    nc.compile()
    return nc


def _get_nc():
    if "nc" not in _CACHE:
        _CACHE["nc"] = _build()
    return _CACHE["nc"]


def _make_in_maps(x, delta_times, valid_mask, w, beta):
    w64 = w.astype(np.float64)
    wsm = np.exp(w64 - w64.max())
    wsm /= wsm.sum()
    b = 1.0 / (1.0 + np.exp(-float(beta[0])))
    bwsm = np.tile((b / (1.0 - b) * wsm)[None, :], (128, 1)).astype(np.float32)
    ident = np.eye(128, dtype=np.float32)
    wz = np.zeros(GB * WBLK, np.float32).astype(ml_dtypes.bfloat16)

    in_maps = []
    for i in range(B):
        xp = np.zeros((NPAD, D), np.float32)
        xp[:N] = x[i]
        dtp = np.zeros(NPAD, np.float32)
        dtp[:N] = delta_times[i]
        vfp = np.zeros(NPAD, np.float32)
        vfp[:N] = valid_mask[i].astype(np.float32)
        m = {
            "x": xp.astype(ml_dtypes.bfloat16),
            "dt": dtp,
            "vf": vfp,
            "bwsm": bwsm,
            "idf": ident,
        }
        for g in range(G):
            m[f"wz{g}"] = wz
        in_maps.append(m)
    return in_maps


def _execute(in_maps, trace=False, **kw):
    nc = _get_nc()
    return run_bass_kernel_spmd(nc, in_maps, core_ids=list(range(B)),
                                trace=trace, **kw)


def kernel(x, delta_times, valid_mask, w, beta):
    in_maps = _make_in_maps(x, delta_times, valid_mask, w, beta)
    kr = _execute(in_maps, trace=False)
    outs = [kr.results[i]["out"][:N].astype(np.float32) for i in range(B)]
    return np.stack(outs, axis=0)


# revision 16
# speedup vs baseline: 1.4686x; 1.0034x over previous
"""AdaptiveTokenMixer Trainium2 kernel (8 NeuronCores, pure data parallel).

Per-core algorithm (one batch element per core), pipelined over 5 groups of
7 position-blocks (BLK=120 outputs per block):
  1. alpha stage: delta_times/valid_mask loaded as per-block rows; sliding
     windows formed by PE transposes of free-dim-shifted slices (t1 =
     BIG - dt fused into the PSUM evictions; vf path in bf16); cv/lg
     computed per-strip under the transpose stream; masked temporal-decay
     softmax over K=8 offsets (elementwise ops split ~60/40 across
     vector/gpsimd) blended with host-precomputed (b/(1-b))*softmax(w);
     alpha finalized per group -> af bf16.
  2. W stage (per group): af group-slice written to a per-group DRAM
     scratch with a SKEWED access pattern (banded W^T[m, k] =
     alpha[n0+m, k-m], m-major 128x128 tiles over a zeros-initialized
     buffer); loaded back with an XBAR-transposing DMA directly into
     W[k, m] orientation.
  3. Mix (per block): out[m, :] = sum_k W[k, m] * x[n0+k, :] -- one
     128x128 @ 128x256 bf16 matmul per block (PSUM f32), two blocks per
     PSUM bank.
  4. Evict PSUM -> SBUF bf16 (paired, alternating vector/scalar), per-group
     DMA store overlapping later groups' matmuls.

Self-contained: hardcodes shapes for B=8, N=4096, d=256, K=8.
"""
import numpy as np
import ml_dtypes

import concourse.bass as bass
import concourse.bacc as bacc
import concourse.mybir as mybir
from concourse import tile
from concourse.bass_utils import run_bass_kernel_spmd

B, N, D, K = 8, 4096, 256, 8
BLK = 120                      # output positions per block
NB = (N + BLK - 1) // BLK      # 35 blocks -> covers 4200 positions
NOUT = NB * BLK                # 4200 rows in padded device output
NPAD = 4224                    # padded input length (>= 34*120 + 136)
KW = 128                       # k-window (contraction) per block
WBLK = KW * KW                 # W scratch elements per block
F = K * NB                     # alpha free size (b-major, p-minor)
BIG = 1024.0
G = 5                          # pipeline groups
GB = NB // G                   # blocks per group (7)

_CACHE = {}


def _build():
    nc = bacc.Bacc("TRN2", target_bir_lowering=False, debug=False,
                   num_devices=B)
    f32 = mybir.dt.float32
    bf16 = mybir.dt.bfloat16

    x_t = nc.dram_tensor("x", [NPAD, D], bf16, kind="ExternalInput")
    dt_t = nc.dram_tensor("dt", [NPAD], f32, kind="ExternalInput")
    vf_t = nc.dram_tensor("vf", [NPAD], bf16, kind="ExternalInput")
    bwsm_t = nc.dram_tensor("bwsm", [128, K], f32, kind="ExternalInput")
    idf_t = nc.dram_tensor("idf", [128, 128], f32, kind="ExternalInput")
    idb_t = nc.dram_tensor("idb", [128, 128], bf16, kind="ExternalInput")
    wz_t = [nc.dram_tensor(f"wz{g}", [GB * WBLK], bf16, kind="ExternalInput")
            for g in range(G)]
    out_t = nc.dram_tensor("out", [NOUT, D], bf16, kind="ExternalOutput")

    def pb(t):  # [128,(b,p)] view -> [128, b, p] (p innermost, for reduces)
        return bass.AP(t.tensor, t.offset, [t.ap[0], [K, NB], [1, K]])

    def strip(t, p):  # tap-p strip [128, NB] (stride K)
        return bass.AP(t.tensor, t.offset + p, [t.ap[0], [K, NB]])

    def strip_h(t, p, b0, nb):  # block range of a tap strip
        return bass.AP(t.tensor, t.offset + p + b0 * K, [t.ap[0], [K, nb]])

    def exp_nb_h(a, b0, nb):  # block-range slice of an exp_nb broadcast
        return bass.AP(a.tensor, a.offset + b0 * a.ap[1][0],
                       [a.ap[0], [a.ap[1][0], nb], [0, K]])

    def exp_k_h(a, nb):       # block-range slice of an exp_k broadcast
        return bass.AP(a.tensor, a.offset, [a.ap[0], [0, nb], [1, K]])

    NB1 = 21                  # block split for vector/gpsimd halves (3 grp)
    NB2 = NB - NB1
    F1 = NB1 * K

    def tt2(out, a, b2, op, b2h=None):
        # tensor_tensor split across vector/gpsimd at block boundary NB1.
        if b2h is None:
            b2h = (b2[:, :F1], b2[:, F1:])
        nc.vector.tensor_tensor(out[:, :F1], a[:, :F1], b2h[0], op)
        nc.gpsimd.tensor_tensor(out[:, F1:], a[:, F1:], b2h[1], op)

    with tile.TileContext(nc) as tc:
        with tc.tile_pool(name="alph", bufs=1) as apool, \
             tc.tile_pool(name="outg", bufs=3) as opool, \
             tc.tile_pool(name="big", bufs=1) as bpool, \
             tc.tile_pool(name="psA", bufs=2, space="PSUM") as psA, \
             tc.tile_pool(name="psB", bufs=3, space="PSUM") as psB:

            # ---- input / constant loads (sync queue) ----
            dt_rows = apool.tile([35, 136], f32)
            nc.sync.dma_start(dt_rows[:],
                              bass.AP(dt_t, 0, [[BLK, NB], [1, 136]]))
            ident_f = apool.tile([128, 128], f32)
            nc.sync.dma_start(ident_f[:], idf_t.ap())
            vf_rows = apool.tile([35, 136], bf16)
            nc.sync.dma_start(vf_rows[:],
                              bass.AP(vf_t, 0, [[BLK, NB], [1, 136]]))
            ident_b = apool.tile([128, 128], bf16)
            nc.sync.dma_start(ident_b[:], idb_t.ap())
            bwsm = apool.tile([128, K], f32)
            nc.sync.dma_start(bwsm[:], bwsm_t.ap())
            # x windows, loaded per group: x_all[i, b, d] = x[b*120+i, d]
            x_all = bpool.tile([128, NB, D], bf16)
            for g in range(G):
                nc.sync.dma_start(
                    x_all[:, g * GB:(g + 1) * GB, :],
                    bass.AP(x_t, g * GB * BLK * D,
                            [[D, 128], [BLK * D, GB], [1, D]]))

            # ---- window strips via PE transpose; cv/lg fused in ----
            # t1 = BIG - dt window (fused into the dt PSUM evictions).
            t1 = apool.tile([128, F], f32)
            vw = apool.tile([128, F], bf16)
            cv = apool.tile([128, F], f32)
            lg = apool.tile([128, F], f32)
            for p in range(K):
                ptd = psA.tile([128, NB], f32, tag="wind")
                nc.tensor.transpose(ptd[:], dt_rows[:NB, p:p + 128],
                                    ident_f[:NB, :NB])
                ptv = psA.tile([128, NB], bf16, tag="winv")
                nc.tensor.transpose(ptv[:], vf_rows[:NB, p:p + 128],
                                    ident_b[:NB, :NB])
                nc.scalar.activation(strip(t1, p), ptd[:],
                                     mybir.ActivationFunctionType.Copy,
                                     bias=BIG, scale=-1.0)
                nc.vector.tensor_copy(strip(vw, p), ptv[:])
                # cv_p = vw_p * vw_0; lg_p = t1_p * cv_p (under the stream)
                nc.vector.tensor_tensor(strip(cv, p), strip(vw, p),
                                        strip(vw, 0), mybir.AluOpType.mult)
                nc.gpsimd.tensor_tensor(strip(lg, p), strip(t1, p),
                                        strip(cv, p), mybir.AluOpType.mult)

            # ---- alpha stage ----
            mx = apool.tile([128, NB], f32)
            nc.vector.tensor_reduce(mx[:], pb(lg), mybir.AxisListType.X,
                                    mybir.AluOpType.max)
            ei = apool.tile([128, F], f32)
            tt2(ei, lg, None, mybir.AluOpType.subtract,
                b2h=(exp_nb_h(mx[:, :], 0, NB1), exp_nb_h(mx[:, :], NB1, NB2)))
            e = apool.tile([128, F], f32)
            nc.scalar.activation(e[:], ei[:], mybir.ActivationFunctionType.Exp)
            s = apool.tile([128, NB], f32)
            nc.vector.tensor_reduce(s[:], pb(e), mybir.AxisListType.X,
                                    mybir.AluOpType.add)
            rcp = apool.tile([128, NB], f32)
            nc.vector.reciprocal(rcp[:], s[:])
            th = apool.tile([128, F], f32)
            tt2(th, e, None, mybir.AluOpType.mult,
                b2h=(exp_nb_h(rcp[:, :], 0, NB1),
                     exp_nb_h(rcp[:, :], NB1, NB2)))
            au = apool.tile([128, F], f32)
            tt2(au, th, None, mybir.AluOpType.add,
                b2h=(exp_k_h(bwsm[:, :], NB1), exp_k_h(bwsm[:, :], NB2)))
            tt2(au, au, cv, mybir.AluOpType.mult)
            sa = apool.tile([128, NB], f32)
            nc.vector.tensor_reduce(sa[:], pb(au), mybir.AxisListType.X,
                                    mybir.AluOpType.add)
            nc.vector.tensor_scalar(sa[:], sa[:], 1e-8, None,
                                    mybir.AluOpType.max)
            r = apool.tile([128, NB], f32)
            nc.vector.reciprocal(r[:], sa[:])
            nc.vector.tensor_tensor(r[:], r[:], strip(vw, 0),
                                    mybir.AluOpType.mult)
            # finalize alpha per group so skew_g starts early
            af = apool.tile([128, F], bf16)
            for g in range(G):
                b0 = g * GB
                eng = nc.vector if g % 2 == 0 else nc.gpsimd
                eng.tensor_tensor(
                    af[:, b0 * K:(b0 + GB) * K],
                    au[:, b0 * K:(b0 + GB) * K],
                    exp_nb_h(r[:, :], b0, GB),
                    mybir.AluOpType.mult)

            # ---- skewed W writes (per group, sync): W^T[m, m+p] banded ----
            for g in range(G):
                nc.sync.dma_start(
                    bass.AP(wz_t[g], 0, [[KW + 1, BLK], [WBLK, GB], [1, K]]),
                    bass.AP(af.tensor, af.offset + g * GB * K,
                            [af.ap[0], [K, GB], [1, K]])[:BLK, :, :])

            # ---- XBAR-transposed load back (per group, scalar) ----
            w_all = bpool.tile([128, NB, KW], bf16)
            for g in range(G):
                nc.scalar.dma_start(
                    w_all[:, g * GB:(g + 1) * GB, :],
                    bass.AP(wz_t[g], 0, [[KW, GB * KW], [1, KW]]),
                    transpose=True)

            # ---- per-block banded matmul; paired evict; per-group store ----
            for g in range(G):
                out_g = opool.tile([128, GB, D], bf16, tag="og")
                for j in range(0, GB, 2):
                    b = g * GB + j
                    npair = min(2, GB - j)
                    pt = psB.tile([KW, 2 * D], f32, tag="mm")
                    for q in range(npair):
                        nc.tensor.matmul(pt[:, q * D:(q + 1) * D],
                                         w_all[:, b + q, :],
                                         x_all[:, b + q, :])
                    if (j // 2) % 2 == 0:
                        nc.vector.tensor_copy(out_g[:BLK, j:j + npair, :],
                                              pt[:BLK, :npair * D])
                    else:
                        nc.scalar.copy(out_g[:BLK, j:j + npair, :],
                                       pt[:BLK, :npair * D])
                nc.sync.dma_start(
                    bass.AP(out_t, g * GB * BLK * D,
                            [[D, BLK], [BLK * D, GB], [1, D]]),
                    out_g[:BLK, :, :])
    nc.compile()
    return nc


def _get_nc():
    if "nc" not in _CACHE:
        _CACHE["nc"] = _build()
    return _CACHE["nc"]


def _make_in_maps(x, delta_times, valid_mask, w, beta):
    w64 = w.astype(np.float64)
    wsm = np.exp(w64 - w64.max())
    wsm /= wsm.sum()
    b = 1.0 / (1.0 + np.exp(-float(beta[0])))
    bwsm = np.tile((b / (1.0 - b) * wsm)[None, :], (128, 1)).astype(np.float32)
    ident = np.eye(128, dtype=np.float32)
    wz = np.zeros(GB * WBLK, np.float32).astype(ml_dtypes.bfloat16)

    in_maps = []
    for i in range(B):
        xp = np.zeros((NPAD, D), np.float32)
        xp[:N] = x[i]
        dtp = np.zeros(NPAD, np.float32)
        dtp[:N] = delta_times[i]
        vfp = np.zeros(NPAD, np.float32)
        vfp[:N] = valid_mask[i].astype(np.float32)
        m = {
            "x": xp.astype(ml_dtypes.bfloat16),
            "dt": dtp,
            "vf": vfp.astype(ml_dtypes.bfloat16),
            "bwsm": bwsm,
            "idf": ident,
            "idb": ident.astype(ml_dtypes.bfloat16),
        }
        for g in range(G):
            m[f"wz{g}"] = wz
        in_maps.append(m)
    return in_maps


def _execute(in_maps, trace=False, **kw):
    nc = _get_nc()
    return run_bass_kernel_spmd(nc, in_maps, core_ids=list(range(B)),
                                trace=trace, **kw)


def kernel(x, delta_times, valid_mask, w, beta):
    in_maps = _make_in_maps(x, delta_times, valid_mask, w, beta)
    kr = _execute(in_maps, trace=False)
    outs = [kr.results[i]["out"][:N].astype(np.float32) for i in range(B)]
    return np.stack(outs, axis=0)


# revision 22
# speedup vs baseline: 1.4737x; 1.0035x over previous
"""AdaptiveTokenMixer Trainium2 kernel (8 NeuronCores, pure data parallel).

Per-core algorithm (one batch element per core), pipelined over 2 chunks
(18+17 position-blocks of BLK=120 outputs) mapped to the two HWDGE rings:
  1. alpha stage: delta_times/valid_mask host-packed into one [70, 136]
     row tensor; 8 PE transposes (one per tap) produce both windows; t1 =
     BIG - dt fused into the scalar-engine PSUM evictions; cv/lg computed
     per-strip under the transpose stream; masked temporal-decay softmax
     over K=8 offsets (elementwise split across vector/gpsimd), blended as
     au = (e + s*c)*cv (scale-invariant rewrite avoids the reciprocal);
     alpha finalized per chunk -> af bf16.
  2. W stage (per chunk): af chunk written to a DRAM scratch with a SKEWED
     access pattern (banded W^T[m, k] = alpha[n0+m, k-m], m-major 128x128
     tiles over a zeros-initialized buffer); loaded back with an
     XBAR-transposing DMA into W[k, m] orientation. Chunk A goes through
     the Act ring (scalar), chunk B through the SP ring (sync) to avoid
     HWDGE wait coalescing between the two loads.
  3. Mix (per block): out[m, :] = sum_k W[k, m] * x[n0+k, :] -- one
     128x128 @ 128x256 bf16 matmul per block (PSUM f32), two blocks per
     PSUM bank.
  4. Evict PSUM -> SBUF bf16 (paired, alternating vector/scalar), 5
     group-stores overlapping later matmuls.

Self-contained: hardcodes shapes for B=8, N=4096, d=256, K=8.
"""
import numpy as np
import ml_dtypes

import concourse.bass as bass
import concourse.bacc as bacc
import concourse.mybir as mybir
from concourse import tile
from concourse.bass_utils import run_bass_kernel_spmd

B, N, D, K = 8, 4096, 256, 8
BLK = 120                      # output positions per block
NB = (N + BLK - 1) // BLK      # 35 blocks -> covers 4200 positions
NOUT = NB * BLK                # 4200 rows in padded device output
NPAD = 4224                    # padded input length (>= 34*120 + 136)
KW = 128                       # k-window (contraction) per block
WBLK = KW * KW                 # W scratch elements per block
F = K * NB                     # alpha free size (b-major, p-minor)
BIG = 1024.0
CA = 18                        # chunk A blocks (Act ring)
CB = NB - CA                   # chunk B blocks (SP ring)
G = 5                          # store groups
GB = NB // G                   # blocks per store group (7)

_CACHE = {}


def _build():
    nc = bacc.Bacc("TRN2", target_bir_lowering=False, debug=False,
                   num_devices=B)
    f32 = mybir.dt.float32
    bf16 = mybir.dt.bfloat16

    x_t = nc.dram_tensor("x", [NPAD, D], bf16, kind="ExternalInput")
    dvf_t = nc.dram_tensor("dvf", [70, 136], f32, kind="ExternalInput")
    vfr_t = nc.dram_tensor("vfr", [35, 136], f32, kind="ExternalInput")
    bwsm_t = nc.dram_tensor("bwsm", [128, K], f32, kind="ExternalInput")
    idf_t = nc.dram_tensor("idf", [128, 128], f32, kind="ExternalInput")
    wza_t = nc.dram_tensor("wza", [CA * WBLK], bf16, kind="ExternalInput")
    wzb_t = nc.dram_tensor("wzb", [CB * WBLK], bf16, kind="ExternalInput")
    out_t = nc.dram_tensor("out", [NOUT, D], bf16, kind="ExternalOutput")

    def pb(t):  # [128,(b,p)] view -> [128, b, p] (p innermost, for reduces)
        return bass.AP(t.tensor, t.offset, [t.ap[0], [K, NB], [1, K]])

    def strip(t, p):  # tap-p strip [128, NB] (stride K)
        return bass.AP(t.tensor, t.offset + p, [t.ap[0], [K, NB]])

    def exp_nb_h(a, b0, nb):  # block-range slice of an exp_nb broadcast
        return bass.AP(a.tensor, a.offset + b0 * a.ap[1][0],
                       [a.ap[0], [a.ap[1][0], nb], [0, K]])

    def exp_k_h(a, nb):       # block-range slice of an exp_k broadcast
        return bass.AP(a.tensor, a.offset, [a.ap[0], [0, nb], [1, K]])

    NB1 = 24                  # block split for vector/gpsimd halves
    NB2 = NB - NB1
    F1 = NB1 * K

    def tt2(out, a, b2, op, b2h=None, ah=None):
        # tensor_tensor split across vector/gpsimd at block boundary NB1.
        if b2h is None:
            b2h = (b2[:, :F1], b2[:, F1:])
        if ah is None:
            ah = (a[:, :F1], a[:, F1:])
        nc.vector.tensor_tensor(out[:, :F1], ah[0], b2h[0], op)
        nc.gpsimd.tensor_tensor(out[:, F1:], ah[1], b2h[1], op)

    with tile.TileContext(nc) as tc:
        with tc.tile_pool(name="alph", bufs=1) as apool, \
             tc.tile_pool(name="outg", bufs=3) as opool, \
             tc.tile_pool(name="big", bufs=1) as bpool, \
             tc.tile_pool(name="psA", bufs=2, space="PSUM") as psA, \
             tc.tile_pool(name="psB", bufs=3, space="PSUM") as psB:

            # ---- input / constant loads (sync = SP ring) ----
            dvf = apool.tile([70, 136], f32)
            nc.sync.dma_start(dvf[:], dvf_t.ap())
            vfr = apool.tile([35, 136], f32)
            nc.sync.dma_start(vfr[:], vfr_t.ap())
            ident_f = apool.tile([128, 128], f32)
            nc.sync.dma_start(ident_f[:], idf_t.ap())
            bwsm = apool.tile([128, K], f32)
            nc.sync.dma_start(bwsm[:], bwsm_t.ap())
            # x windows in two chunks: x_all[i, b, d] = x[b*120+i, d]
            x_all = bpool.tile([128, NB, D], bf16)
            for c0, cn in ((0, CA), (CA, CB)):
                nc.sync.dma_start(
                    x_all[:, c0:c0 + cn, :],
                    bass.AP(x_t, c0 * BLK * D,
                            [[D, 128], [BLK * D, cn], [1, D]]))

            # ---- window strips: one [70,128] PE transpose per tap ----
            # rows 0..34 = dt blocks, rows 35..69 = vf blocks.
            # t1 = BIG - dt fused into the scalar PSUM eviction.
            t1 = apool.tile([128, F], f32)
            vw = apool.tile([128, F], f32)
            cv = apool.tile([128, F], f32)
            lg = apool.tile([128, F], f32)
            for p in range(K):
                ptw = psA.tile([128, 70], f32, tag="win")
                nc.tensor.transpose(ptw[:, 0:NB], dvf[:NB, p:p + 128],
                                    ident_f[:NB, :NB])
                nc.tensor.transpose(ptw[:, NB:70], vfr[:NB, p:p + 128],
                                    ident_f[:NB, :NB])
                nc.scalar.activation(strip(t1, p), ptw[:, 0:NB],
                                     mybir.ActivationFunctionType.Copy,
                                     bias=BIG, scale=-1.0)
                nc.vector.tensor_copy(strip(vw, p), ptw[:, NB:70])
                # cv_p = vw_p * vw_0; lg_p = t1_p * cv_p (under the stream)
                nc.vector.tensor_tensor(strip(cv, p), strip(vw, p),
                                        strip(vw, 0), mybir.AluOpType.mult)
                nc.gpsimd.tensor_tensor(strip(lg, p), strip(t1, p),
                                        strip(cv, p), mybir.AluOpType.mult)

            # ---- alpha stage ----
            mx = apool.tile([128, NB], f32)
            nc.vector.tensor_reduce(mx[:], pb(lg), mybir.AxisListType.X,
                                    mybir.AluOpType.max)
            ei = apool.tile([128, F], f32)
            tt2(ei, lg, None, mybir.AluOpType.subtract,
                b2h=(exp_nb_h(mx[:, :], 0, NB1), exp_nb_h(mx[:, :], NB1, NB2)))
            e = apool.tile([128, F], f32)
            nc.scalar.activation(e[:], ei[:], mybir.ActivationFunctionType.Exp)
            s = apool.tile([128, NB], f32)
            nc.vector.tensor_reduce(s[:], pb(e), mybir.AxisListType.X,
                                    mybir.AluOpType.add)
            rcp = apool.tile([128, NB], f32)
            nc.vector.reciprocal(rcp[:], s[:])
            th = apool.tile([128, F], f32)
            tt2(th, e, None, mybir.AluOpType.mult,
                b2h=(exp_nb_h(rcp[:, :], 0, NB1),
                     exp_nb_h(rcp[:, :], NB1, NB2)))
            au = apool.tile([128, F], f32)
            tt2(au, th, None, mybir.AluOpType.add,
                b2h=(exp_k_h(bwsm[:, :], NB1), exp_k_h(bwsm[:, :], NB2)))
            tt2(au, au, cv, mybir.AluOpType.mult)
            sa = apool.tile([128, NB], f32)
            nc.vector.tensor_reduce(sa[:], pb(au), mybir.AxisListType.X,
                                    mybir.AluOpType.add)
            nc.vector.tensor_scalar(sa[:], sa[:], 1e-8, None,
                                    mybir.AluOpType.max)
            r = apool.tile([128, NB], f32)
            nc.vector.reciprocal(r[:], sa[:])
            nc.vector.tensor_tensor(r[:], r[:], strip(vw, 0),
                                    mybir.AluOpType.mult)
            # finalize alpha per chunk so skews start early
            af = apool.tile([128, F], bf16)
            nc.vector.tensor_tensor(af[:, :CA * K], au[:, :CA * K],
                                    exp_nb_h(r[:, :], 0, CA),
                                    mybir.AluOpType.mult)
            nc.gpsimd.tensor_tensor(af[:, CA * K:], au[:, CA * K:],
                                    exp_nb_h(r[:, :], CA, CB),
                                    mybir.AluOpType.mult)

            # ---- skewed W writes (sync): W^T[b][m, m+p] = af[m, p, b] ----
            for wt, c0, cn in ((wza_t, 0, CA), (wzb_t, CA, CB)):
                nc.sync.dma_start(
                    bass.AP(wt, 0, [[KW + 1, BLK], [WBLK, cn], [1, K]]),
                    bass.AP(af.tensor, af.offset + c0 * K,
                            [af.ap[0], [K, cn], [1, K]])[:BLK, :, :])

            # ---- XBAR-transposed loads: A on Act ring, B on SP ring ----
            # (pieces of <= 7 blocks: larger single transposes corrupt)
            w_all = bpool.tile([128, NB, KW], bf16)
            for c0 in range(0, CA, 7):
                cn = min(7, CA - c0)
                nc.scalar.dma_start(
                    w_all[:, c0:c0 + cn, :],
                    bass.AP(wza_t, c0 * WBLK, [[KW, cn * KW], [1, KW]]),
                    transpose=True)
            for c0 in range(0, CB, 7):
                cn = min(7, CB - c0)
                nc.scalar.dma_start(
                    w_all[:, CA + c0:CA + c0 + cn, :],
                    bass.AP(wzb_t, c0 * WBLK, [[KW, cn * KW], [1, KW]]),
                    transpose=True)

            # ---- per-block banded matmul; paired evict; group stores ----
            for g in range(G):
                out_g = opool.tile([128, GB, D], bf16, tag="og")
                for j in range(0, GB, 2):
                    b = g * GB + j
                    npair = min(2, GB - j)
                    pt = psB.tile([KW, 2 * D], f32, tag="mm")
                    for q in range(npair):
                        nc.tensor.matmul(pt[:, q * D:(q + 1) * D],
                                         w_all[:, b + q, :],
                                         x_all[:, b + q, :])
                    if (j // 2) % 2 == 0:
                        nc.vector.tensor_copy(out_g[:BLK, j:j + npair, :],
                                              pt[:BLK, :npair * D])
                    else:
                        nc.scalar.copy(out_g[:BLK, j:j + npair, :],
                                       pt[:BLK, :npair * D])
                nc.sync.dma_start(
                    bass.AP(out_t, g * GB * BLK * D,
                            [[D, BLK], [BLK * D, GB], [1, D]]),
                    out_g[:BLK, :, :])
    nc.compile()
    return nc


def _get_nc():
    if "nc" not in _CACHE:
        _CACHE["nc"] = _build()
    return _CACHE["nc"]


def _make_in_maps(x, delta_times, valid_mask, w, beta):
    w64 = w.astype(np.float64)
    wsm = np.exp(w64 - w64.max())
    wsm /= wsm.sum()
    b = 1.0 / (1.0 + np.exp(-float(beta[0])))
    bwsm = np.tile((b / (1.0 - b) * wsm)[None, :], (128, 1)).astype(np.float32)
    ident = np.eye(128, dtype=np.float32)
    wza = np.zeros(CA * WBLK, np.float32).astype(ml_dtypes.bfloat16)
    wzb = np.zeros(CB * WBLK, np.float32).astype(ml_dtypes.bfloat16)

    in_maps = []
    for i in range(B):
        xp = np.zeros((NPAD, D), np.float32)
        xp[:N] = x[i]
        dtp = np.zeros(NPAD, np.float32)
        dtp[:N] = delta_times[i]
        vfp = np.zeros(NPAD, np.float32)
        vfp[:N] = valid_mask[i].astype(np.float32)
        dvf = np.zeros((70, 136), np.float32)
        for bb in range(NB):
            dvf[bb, :] = dtp[bb * BLK:bb * BLK + 136]
            dvf[NB + bb, :] = vfp[bb * BLK:bb * BLK + 136]
        in_maps.append({
            "x": xp.astype(ml_dtypes.bfloat16),
            "dvf": dvf,
            "vfr": dvf[NB:70].copy(),
            "bwsm": bwsm,
            "idf": ident,
            "wza": wza,
            "wzb": wzb,
        })
    return in_maps


def _execute(in_maps, trace=False, **kw):
    nc = _get_nc()
    return run_bass_kernel_spmd(nc, in_maps, core_ids=list(range(B)),
                                trace=trace, **kw)


def kernel(x, delta_times, valid_mask, w, beta):
    in_maps = _make_in_maps(x, delta_times, valid_mask, w, beta)
    kr = _execute(in_maps, trace=False)
    outs = [kr.results[i]["out"][:N].astype(np.float32) for i in range(B)]
    return np.stack(outs, axis=0)


# revision 27
# speedup vs baseline: 1.4911x; 1.0118x over previous
"""AdaptiveTokenMixer Trainium2 kernel (8 NeuronCores, pure data parallel).

Per-core algorithm (one batch element per core), pipelined over 2 chunks
(18+17 position-blocks of BLK=120 outputs) mapped to the two HWDGE rings:
  1. alpha stage: delta_times/valid_mask host-packed into one [70, 136]
     row tensor; two PE transposes per tap produce both windows; t1 =
     BIG - dt fused into the scalar-engine PSUM evictions; cv/lg computed
     per-strip under the transpose stream; masked temporal-decay softmax
     over K=8 offsets (elementwise split across vector/gpsimd), blended as
     au = (e + s*c)*cv (scale-invariant rewrite avoids the reciprocal);
     alpha finalized per chunk -> af bf16.
  2. W stage (per chunk): af chunk written to a DRAM scratch with a SKEWED
     access pattern (banded W^T[m, k] = alpha[n0+m, k-m], m-major 128x128
     tiles over a zeros-initialized buffer); loaded back with an
     XBAR-transposing DMA into W[k, m] orientation. The skew and its XBAR
     load MUST be issued on opposite HWDGE rings (SP=sync / Act=scalar):
     a same-ring consumer's semaphore wait is elided under the ring-FIFO
     assumption, but the XBAR read races the skew's multi-engine
     descriptor drain (observed nondeterministic corruption).
  3. Mix (per block): out[m, :] = sum_k W[k, m] * x[n0+k, :] -- one
     128x128 @ 128x256 bf16 matmul per block (PSUM f32), two blocks per
     PSUM bank.
  4. Evict PSUM -> SBUF bf16 (paired, alternating vector/scalar), 5
     group-stores overlapping later matmuls.

Self-contained: hardcodes shapes for B=8, N=4096, d=256, K=8.
"""
import numpy as np
import ml_dtypes

import concourse.bass as bass
import concourse.bacc as bacc
import concourse.mybir as mybir
from concourse import tile
from concourse.bass_utils import run_bass_kernel_spmd

B, N, D, K = 8, 4096, 256, 8
BLK = 120                      # output positions per block
NB = (N + BLK - 1) // BLK      # 35 blocks -> covers 4200 positions
NOUT = NB * BLK                # 4200 rows in padded device output
NPAD = 4224                    # padded input length (>= 34*120 + 136)
KW = 128                       # k-window (contraction) per block
WBLK = KW * KW                 # W scratch elements per block
F = K * NB                     # alpha free size (b-major, p-minor)
BIG = 1024.0
CA = 18                        # chunk A blocks (skew on SP, load on Act)
CB = NB - CA                   # chunk B blocks (skew on Act, load on SP)
G = 5                          # store groups
GB = NB // G                   # blocks per store group (7)

_CACHE = {}


def _build():
    nc = bacc.Bacc("TRN2", target_bir_lowering=False, debug=False,
                   num_devices=B)
    f32 = mybir.dt.float32
    bf16 = mybir.dt.bfloat16

    x_t = nc.dram_tensor("x", [NPAD, D], bf16, kind="ExternalInput")
    dvf_t = nc.dram_tensor("dvf", [99, 136], f32, kind="ExternalInput")
    vfr_t = nc.dram_tensor("vfr", [35, 136], f32, kind="ExternalInput")
    bwsm_t = nc.dram_tensor("bwsm", [128, K], f32, kind="ExternalInput")
    idf_t = nc.dram_tensor("idf", [128, 128], f32, kind="ExternalInput")
    wza_t = nc.dram_tensor("wza", [CA * WBLK], bf16, kind="ExternalInput")
    wzb_t = nc.dram_tensor("wzb", [CB * WBLK], bf16, kind="ExternalInput")
    out_t = nc.dram_tensor("out", [NOUT, D], bf16, kind="ExternalOutput")

    def pb(t):  # [128,(b,p)] view -> [128, b, p] (p innermost, for reduces)
        return bass.AP(t.tensor, t.offset, [t.ap[0], [K, NB], [1, K]])

    def strip(t, p):  # tap-p strip [128, NB] (stride K)
        return bass.AP(t.tensor, t.offset + p, [t.ap[0], [K, NB]])

    def exp_nb_h(a, b0, nb):  # block-range slice of an exp_nb broadcast
        return bass.AP(a.tensor, a.offset + b0 * a.ap[1][0],
                       [a.ap[0], [a.ap[1][0], nb], [0, K]])

    def exp_k_h(a, nb):       # block-range slice of an exp_k broadcast
        return bass.AP(a.tensor, a.offset, [a.ap[0], [0, nb], [1, K]])

    NB1 = 24                  # block split for vector/gpsimd halves
    NB2 = NB - NB1
    F1 = NB1 * K

    def tt2(out, a, b2, op, b2h=None, ah=None):
        # tensor_tensor split across vector/gpsimd at block boundary NB1.
        if b2h is None:
            b2h = (b2[:, :F1], b2[:, F1:])
        if ah is None:
            ah = (a[:, :F1], a[:, F1:])
        nc.vector.tensor_tensor(out[:, :F1], ah[0], b2h[0], op)
        nc.gpsimd.tensor_tensor(out[:, F1:], ah[1], b2h[1], op)

    with tile.TileContext(nc) as tc:
        with tc.tile_pool(name="alph", bufs=1) as apool, \
             tc.tile_pool(name="outg", bufs=3) as opool, \
             tc.tile_pool(name="big", bufs=1) as bpool, \
             tc.tile_pool(name="psA", bufs=2, space="PSUM") as psA, \
             tc.tile_pool(name="psB", bufs=3, space="PSUM") as psB:

            # ---- input / constant loads (sync = SP ring) ----
            dvf = apool.tile([99, 136], f32)
            nc.sync.dma_start(dvf[:], dvf_t.ap())
            vfr = apool.tile([35, 136], f32)
            nc.sync.dma_start(vfr[:], vfr_t.ap())
            ident_f = apool.tile([128, 128], f32)
            nc.sync.dma_start(ident_f[:], idf_t.ap())
            bwsm = apool.tile([128, K], f32)
            nc.sync.dma_start(bwsm[:], bwsm_t.ap())
            # x windows in two chunks: x_all[i, b, d] = x[b*120+i, d]
            x_all = bpool.tile([128, NB, D], bf16)
            for c0, cn in ((0, CA), (CA, CB)):
                nc.sync.dma_start(
                    x_all[:, c0:c0 + cn, :],
                    bass.AP(x_t, c0 * BLK * D,
                            [[D, 128], [BLK * D, cn], [1, D]]))

            # ---- window strips: two PE transposes per tap ----
            # dvf rows 0..34 = dt blocks, rows 64..98 = vf blocks.
            # t1 = BIG - dt fused into the scalar PSUM eviction.
            t1 = apool.tile([128, F], f32)
            vw = apool.tile([128, F], f32)
            cv = apool.tile([128, F], f32)
            lg = apool.tile([128, F], f32)
            for p in range(K):
                ptw = psA.tile([128, 70], f32, tag="win")
                nc.tensor.transpose(ptw[:, 0:NB], dvf[:NB, p:p + 128],
                                    ident_f[:NB, :NB])
                nc.tensor.transpose(ptw[:, NB:70], vfr[:NB, p:p + 128],
                                    ident_f[:NB, :NB])
                nc.scalar.activation(strip(t1, p), ptw[:, 0:NB],
                                     mybir.ActivationFunctionType.Copy,
                                     bias=BIG, scale=-1.0)
                nc.vector.tensor_copy(strip(vw, p), ptw[:, NB:70])
                # cv_p = vw_p * vw_0; lg_p = t1_p * cv_p (under the stream)
                nc.vector.tensor_tensor(strip(cv, p), strip(vw, p),
                                        strip(vw, 0), mybir.AluOpType.mult)
                nc.gpsimd.tensor_tensor(strip(lg, p), strip(t1, p),
                                        strip(cv, p), mybir.AluOpType.mult)

            # ---- alpha stage ----
            mx = apool.tile([128, NB], f32)
            nc.vector.tensor_reduce(mx[:], pb(lg), mybir.AxisListType.X,
                                    mybir.AluOpType.max)
            ei = apool.tile([128, F], f32)
            tt2(ei, lg, None, mybir.AluOpType.subtract,
                b2h=(exp_nb_h(mx[:, :], 0, NB1), exp_nb_h(mx[:, :], NB1, NB2)))
            e = apool.tile([128, F], f32)
            nc.scalar.activation(e[:], ei[:], mybir.ActivationFunctionType.Exp)
            s = apool.tile([128, NB], f32)
            nc.vector.tensor_reduce(s[:], pb(e), mybir.AxisListType.X,
                                    mybir.AluOpType.add)
            # scale-invariant blend: au = (e + s*c) * cv  (c = bwsm row)
            sc = apool.tile([128, F], f32)
            tt2(sc, None, None, mybir.AluOpType.mult,
                ah=(exp_nb_h(s[:, :], 0, NB1), exp_nb_h(s[:, :], NB1, NB2)),
                b2h=(exp_k_h(bwsm[:, :], NB1), exp_k_h(bwsm[:, :], NB2)))
            au = apool.tile([128, F], f32)
            tt2(au, sc, e, mybir.AluOpType.add)
            tt2(au, au, cv, mybir.AluOpType.mult)
            sa = apool.tile([128, NB], f32)
            nc.vector.tensor_reduce(sa[:], pb(au), mybir.AxisListType.X,
                                    mybir.AluOpType.add)
            nc.vector.tensor_scalar(sa[:], sa[:], 1e-8, None,
                                    mybir.AluOpType.max)
            r = apool.tile([128, NB], f32)
            nc.vector.reciprocal(r[:], sa[:])
            nc.vector.tensor_tensor(r[:], r[:], strip(vw, 0),
                                    mybir.AluOpType.mult)
            # finalize alpha per chunk so skews start early
            af = apool.tile([128, F], bf16)
            nc.vector.tensor_tensor(af[:, :CA * K], au[:, :CA * K],
                                    exp_nb_h(r[:, :], 0, CA),
                                    mybir.AluOpType.mult)
            nc.gpsimd.tensor_tensor(af[:, CA * K:], au[:, CA * K:],
                                    exp_nb_h(r[:, :], CA, CB),
                                    mybir.AluOpType.mult)

            # ---- skewed W writes: W^T[b][m, m+p] = af[m, p, b] ----
            # chunk A skew on SP ring, chunk B skew on Act ring
            nc.sync.dma_start(
                bass.AP(wza_t, 0, [[KW + 1, BLK], [WBLK, CA], [1, K]]),
                bass.AP(af.tensor, af.offset,
                        [af.ap[0], [K, CA], [1, K]])[:BLK, :, :])
            nc.sync.dma_start(
                bass.AP(wzb_t, 0, [[KW + 1, BLK], [WBLK, CB], [1, K]]),
                bass.AP(af.tensor, af.offset + CA * K,
                        [af.ap[0], [K, CB], [1, K]])[:BLK, :, :])

            # ---- XBAR-transposed loads (opposite ring from the skew) ----
            w_all = bpool.tile([128, NB, KW], bf16)
            for c0 in range(0, CA, 7):
                cn = min(7, CA - c0)
                nc.scalar.dma_start(
                    w_all[:, c0:c0 + cn, :],
                    bass.AP(wza_t, c0 * WBLK, [[KW, cn * KW], [1, KW]]),
                    transpose=True)
            for c0 in range(0, CB, 7):
                cn = min(7, CB - c0)
                nc.scalar.dma_start(
                    w_all[:, CA + c0:CA + c0 + cn, :],
                    bass.AP(wzb_t, c0 * WBLK, [[KW, cn * KW], [1, KW]]),
                    transpose=True)

            # ---- per-block banded matmul; paired evict; group stores ----
            for g in range(G):
                out_g = opool.tile([128, GB, D], bf16, tag="og")
                for j in range(0, GB, 2):
                    b = g * GB + j
                    npair = min(2, GB - j)
                    pt = psB.tile([KW, 2 * D], f32, tag="mm")
                    for q in range(npair):
                        nc.tensor.matmul(pt[:, q * D:(q + 1) * D],
                                         w_all[:, b + q, :],
                                         x_all[:, b + q, :])
                    if (j // 2) % 2 == 0:
                        nc.vector.tensor_copy(out_g[:BLK, j:j + npair, :],
                                              pt[:BLK, :npair * D])
                    else:
                        nc.scalar.copy(out_g[:BLK, j:j + npair, :],
                                       pt[:BLK, :npair * D])
                nc.sync.dma_start(
                    bass.AP(out_t, g * GB * BLK * D,
                            [[D, BLK], [BLK * D, GB], [1, D]]),
                    out_g[:BLK, :, :])
    nc.compile()
    return nc


def _get_nc():
    if "nc" not in _CACHE:
        _CACHE["nc"] = _build()
    return _CACHE["nc"]


def _make_in_maps(x, delta_times, valid_mask, w, beta):
    w64 = w.astype(np.float64)
    wsm = np.exp(w64 - w64.max())
    wsm /= wsm.sum()
    b = 1.0 / (1.0 + np.exp(-float(beta[0])))
    bwsm = np.tile((b / (1.0 - b) * wsm)[None, :], (128, 1)).astype(np.float32)
    ident = np.eye(128, dtype=np.float32)
    wza = np.zeros(CA * WBLK, np.float32).astype(ml_dtypes.bfloat16)
    wzb = np.zeros(CB * WBLK, np.float32).astype(ml_dtypes.bfloat16)

    in_maps = []
    for i in range(B):
        xp = np.zeros((NPAD, D), np.float32)
        xp[:N] = x[i]
        dtp = np.zeros(NPAD, np.float32)
        dtp[:N] = delta_times[i]
        vfp = np.zeros(NPAD, np.float32)
        vfp[:N] = valid_mask[i].astype(np.float32)
        dvf = np.zeros((99, 136), np.float32)
        for bb in range(NB):
            dvf[bb, :] = dtp[bb * BLK:bb * BLK + 136]
            dvf[64 + bb, :] = vfp[bb * BLK:bb * BLK + 136]
        in_maps.append({
            "x": xp.astype(ml_dtypes.bfloat16),
            "dvf": dvf,
            "vfr": dvf[64:99].copy(),
            "bwsm": bwsm,
            "idf": ident,
            "wza": wza,
            "wzb": wzb,
        })
    return in_maps


def _execute(in_maps, trace=False, **kw):
    nc = _get_nc()
    return run_bass_kernel_spmd(nc, in_maps, core_ids=list(range(B)),
                                trace=trace, **kw)


def kernel(x, delta_times, valid_mask, w, beta):
    in_maps = _make_in_maps(x, delta_times, valid_mask, w, beta)
    kr = _execute(in_maps, trace=False)
    outs = [kr.results[i]["out"][:N].astype(np.float32) for i in range(B)]
    return np.stack(outs, axis=0)


# revision 29
# speedup vs baseline: 1.4973x; 1.0041x over previous
"""AdaptiveTokenMixer Trainium2 kernel (8 NeuronCores, pure data parallel).

Per-core algorithm (one batch element per core), pipelined over 2 chunks
(18+17 position-blocks of BLK=120 outputs) mapped to the two HWDGE rings:
  1. alpha stage: delta_times/valid_mask host-packed into one [70, 136]
     row tensor; two PE transposes per tap produce both windows; t1 =
     BIG - dt fused into the scalar-engine PSUM evictions; cv/lg computed
     per-strip under the transpose stream; masked temporal-decay softmax
     over K=8 offsets (elementwise split across vector/gpsimd), blended as
     au = (e + s*c)*cv (scale-invariant rewrite avoids the reciprocal);
     alpha finalized per chunk -> af bf16.
  2. W stage (per chunk): af chunk written to a DRAM scratch with a SKEWED
     access pattern (banded W^T[m, k] = alpha[n0+m, k-m], m-major 128x128
     tiles over a zeros-initialized buffer); loaded back with an
     XBAR-transposing DMA into W[k, m] orientation. The skew and its XBAR
     load MUST be issued on opposite HWDGE rings (SP=sync / Act=scalar):
     a same-ring consumer's semaphore wait is elided under the ring-FIFO
     assumption, but the XBAR read races the skew's multi-engine
     descriptor drain (observed nondeterministic corruption).
  3. Mix (per block): out[m, :] = sum_k W[k, m] * x[n0+k, :] -- one
     128x128 @ 128x256 bf16 matmul per block (PSUM f32), two blocks per
     PSUM bank.
  4. Evict PSUM -> SBUF bf16 (paired, alternating vector/scalar), 5
     group-stores overlapping later matmuls.

Self-contained: hardcodes shapes for B=8, N=4096, d=256, K=8.
"""
import numpy as np
import ml_dtypes

import concourse.bass as bass
import concourse.bacc as bacc
import concourse.mybir as mybir
from concourse import tile
from concourse.bass_utils import run_bass_kernel_spmd

B, N, D, K = 8, 4096, 256, 8
BLK = 120                      # output positions per block
NB = (N + BLK - 1) // BLK      # 35 blocks -> covers 4200 positions
NOUT = NB * BLK                # 4200 rows in padded device output
NPAD = 4224                    # padded input length (>= 34*120 + 136)
KW = 128                       # k-window (contraction) per block
WBLK = KW * KW                 # W scratch elements per block
F = K * NB                     # alpha free size (b-major, p-minor)
BIG = 1024.0
CA = 18                        # chunk A blocks (skew on SP, load on Act)
CB = NB - CA                   # chunk B blocks (skew on Act, load on SP)
G = 5                          # store groups
GB = NB // G                   # blocks per store group (7)

_CACHE = {}


def _build():
    nc = bacc.Bacc("TRN2", target_bir_lowering=False, debug=False,
                   num_devices=B)
    f32 = mybir.dt.float32
    bf16 = mybir.dt.bfloat16

    x_t = nc.dram_tensor("x", [NPAD, D], bf16, kind="ExternalInput")
    dvf_t = nc.dram_tensor("dvf", [99, 136], f32, kind="ExternalInput")
    vfr_t = nc.dram_tensor("vfr", [35, 136], f32, kind="ExternalInput")
    bwsm_t = nc.dram_tensor("bwsm", [128, K], f32, kind="ExternalInput")
    idf_t = nc.dram_tensor("idf", [128, 128], f32, kind="ExternalInput")
    wza_t = nc.dram_tensor("wza", [CA * WBLK], bf16, kind="ExternalInput")
    wzb_t = nc.dram_tensor("wzb", [CB * WBLK], bf16, kind="ExternalInput")
    out_t = nc.dram_tensor("out", [NOUT, D], bf16, kind="ExternalOutput")

    def pb(t):  # [128,(b,p)] view -> [128, b, p] (p innermost, for reduces)
        return bass.AP(t.tensor, t.offset, [t.ap[0], [K, NB], [1, K]])

    def strip(t, p):  # tap-p strip [128, NB] (stride K)
        return bass.AP(t.tensor, t.offset + p, [t.ap[0], [K, NB]])

    def exp_nb_h(a, b0, nb):  # block-range slice of an exp_nb broadcast
        return bass.AP(a.tensor, a.offset + b0 * a.ap[1][0],
                       [a.ap[0], [a.ap[1][0], nb], [0, K]])

    def exp_k_h(a, nb):       # block-range slice of an exp_k broadcast
        return bass.AP(a.tensor, a.offset, [a.ap[0], [0, nb], [1, K]])

    NB1 = 24                  # block split for vector/gpsimd halves
    NB2 = NB - NB1
    F1 = NB1 * K

    def tt2(out, a, b2, op, b2h=None, ah=None):
        # tensor_tensor split across vector/gpsimd at block boundary NB1.
        if b2h is None:
            b2h = (b2[:, :F1], b2[:, F1:])
        if ah is None:
            ah = (a[:, :F1], a[:, F1:])
        nc.vector.tensor_tensor(out[:, :F1], ah[0], b2h[0], op)
        nc.gpsimd.tensor_tensor(out[:, F1:], ah[1], b2h[1], op)

    with tile.TileContext(nc) as tc:
        with tc.tile_pool(name="alph", bufs=1) as apool, \
             tc.tile_pool(name="outg", bufs=3) as opool, \
             tc.tile_pool(name="big", bufs=1) as bpool, \
             tc.tile_pool(name="psA", bufs=2, space="PSUM") as psA, \
             tc.tile_pool(name="psB", bufs=3, space="PSUM") as psB:

            # ---- input / constant loads (sync = SP ring) ----
            dvf = apool.tile([99, 136], f32)
            nc.sync.dma_start(dvf[:], dvf_t.ap())
            vfr = apool.tile([35, 136], f32)
            nc.sync.dma_start(vfr[:], vfr_t.ap())
            ident_f = apool.tile([128, 128], f32)
            nc.sync.dma_start(ident_f[:], idf_t.ap())
            bwsm = apool.tile([128, K], f32)
            nc.sync.dma_start(bwsm[:], bwsm_t.ap())
            # x windows in two chunks: x_all[i, b, d] = x[b*120+i, d]
            x_all = bpool.tile([128, NB, D], bf16)
            for c0, cn in ((0, CA), (CA, CB)):
                nc.sync.dma_start(
                    x_all[:, c0:c0 + cn, :],
                    bass.AP(x_t, c0 * BLK * D,
                            [[D, 128], [BLK * D, cn], [1, D]]))

            # ---- window strips: two PE transposes per tap ----
            # dvf rows 0..34 = dt blocks, rows 64..98 = vf blocks.
            # t1 = BIG - dt fused into the scalar PSUM eviction.
            t1 = apool.tile([128, F], f32)
            vw = apool.tile([128, F], f32)
            cv = apool.tile([128, F], f32)
            lg = apool.tile([128, F], f32)
            for p in range(K):
                ptw = psA.tile([128, 70], f32, tag="win")
                nc.tensor.transpose(ptw[:, 0:NB], dvf[:NB, p:p + 128],
                                    ident_f[:NB, :NB])
                nc.tensor.transpose(ptw[:, NB:70], vfr[:NB, p:p + 128],
                                    ident_f[:NB, :NB])
                nc.scalar.activation(strip(t1, p), ptw[:, 0:NB],
                                     mybir.ActivationFunctionType.Copy,
                                     bias=BIG, scale=-1.0)
                nc.vector.tensor_copy(strip(vw, p), ptw[:, NB:70])
                # cv_p = vw_p * vw_0; lg_p = t1_p * cv_p (under the stream)
                nc.vector.tensor_tensor(strip(cv, p), strip(vw, p),
                                        strip(vw, 0), mybir.AluOpType.mult)
                nc.gpsimd.tensor_tensor(strip(lg, p), strip(t1, p),
                                        strip(cv, p), mybir.AluOpType.mult)

            # ---- alpha stage ----
            mx = apool.tile([128, NB], f32)
            nc.vector.tensor_reduce(mx[:], pb(lg), mybir.AxisListType.X,
                                    mybir.AluOpType.max)
            ei = apool.tile([128, F], f32)
            tt2(ei, lg, None, mybir.AluOpType.subtract,
                b2h=(exp_nb_h(mx[:, :], 0, NB1), exp_nb_h(mx[:, :], NB1, NB2)))
            e = apool.tile([128, F], f32)
            nc.scalar.activation(e[:], ei[:], mybir.ActivationFunctionType.Exp)
            s = apool.tile([128, NB], f32)
            nc.vector.tensor_reduce(s[:], pb(e), mybir.AxisListType.X,
                                    mybir.AluOpType.add)
            # scale-invariant blend: au = (e + s*c) * cv  (c = bwsm row)
            sc = apool.tile([128, F], f32)
            tt2(sc, None, None, mybir.AluOpType.mult,
                ah=(exp_nb_h(s[:, :], 0, NB1), exp_nb_h(s[:, :], NB1, NB2)),
                b2h=(exp_k_h(bwsm[:, :], NB1), exp_k_h(bwsm[:, :], NB2)))
            au = apool.tile([128, F], f32)
            tt2(au, sc, e, mybir.AluOpType.add)
            tt2(au, au, cv, mybir.AluOpType.mult)
            sa = apool.tile([128, NB], f32)
            nc.vector.tensor_reduce(sa[:], pb(au), mybir.AxisListType.X,
                                    mybir.AluOpType.add)
            nc.vector.tensor_scalar(sa[:], sa[:], 1e-8, None,
                                    mybir.AluOpType.max)
            r = apool.tile([128, NB], f32)
            nc.vector.reciprocal(r[:], sa[:])
            nc.vector.tensor_tensor(r[:], r[:], strip(vw, 0),
                                    mybir.AluOpType.mult)
            # finalize alpha per chunk so skews start early
            af = apool.tile([128, F], bf16)
            nc.vector.tensor_tensor(af[:, :CA * K], au[:, :CA * K],
                                    exp_nb_h(r[:, :], 0, CA),
                                    mybir.AluOpType.mult)
            nc.gpsimd.tensor_tensor(af[:, CA * K:], au[:, CA * K:],
                                    exp_nb_h(r[:, :], CA, CB),
                                    mybir.AluOpType.mult)

            # ---- skewed W writes: W^T[b][m, m+p] = af[m, p, b] ----
            # chunk A skew on SP ring, chunk B skew on Act ring
            nc.sync.dma_start(
                bass.AP(wza_t, 0, [[KW + 1, BLK], [WBLK, CA], [1, K]]),
                bass.AP(af.tensor, af.offset,
                        [af.ap[0], [K, CA], [1, K]])[:BLK, :, :])
            nc.sync.dma_start(
                bass.AP(wzb_t, 0, [[KW + 1, BLK], [WBLK, CB], [1, K]]),
                bass.AP(af.tensor, af.offset + CA * K,
                        [af.ap[0], [K, CB], [1, K]])[:BLK, :, :])

            # ---- XBAR-transposed loads (opposite ring from the skew) ----
            # NOTE: the XBAR is one shared unit -- concurrent DMA_TRANSPOSE
            # instructions from the two HWDGE rings corrupt each other.
            # All transposes must serialize on one ring (Act).
            w_all = bpool.tile([128, NB, KW], bf16)
            nc.scalar.dma_start(
                w_all[:, 0:CA, :],
                bass.AP(wza_t, 0, [[KW, CA * KW], [1, KW]]),
                transpose=True)
            nc.scalar.dma_start(
                w_all[:, CA:NB, :],
                bass.AP(wzb_t, 0, [[KW, CB * KW], [1, KW]]),
                transpose=True)

            # ---- per-block banded matmul; paired evict; group stores ----
            for g in range(G):
                out_g = opool.tile([128, GB, D], bf16, tag="og")
                for j in range(0, GB, 2):
                    b = g * GB + j
                    npair = min(2, GB - j)
                    pt = psB.tile([KW, 2 * D], f32, tag="mm")
                    for q in range(npair):
                        nc.tensor.matmul(pt[:, q * D:(q + 1) * D],
                                         w_all[:, b + q, :],
                                         x_all[:, b + q, :])
                    if (j // 2) % 3 == 2:
                        nc.scalar.copy(out_g[:BLK, j:j + npair, :],
                                       pt[:BLK, :npair * D])
                    else:
                        nc.vector.tensor_copy(out_g[:BLK, j:j + npair, :],
                                              pt[:BLK, :npair * D])
                nc.sync.dma_start(
                    bass.AP(out_t, g * GB * BLK * D,
                            [[D, BLK], [BLK * D, GB], [1, D]]),
                    out_g[:BLK, :, :])
    nc.compile()
    return nc


def _get_nc():
    if "nc" not in _CACHE:
        _CACHE["nc"] = _build()
    return _CACHE["nc"]


def _make_in_maps(x, delta_times, valid_mask, w, beta):
    w64 = w.astype(np.float64)
    wsm = np.exp(w64 - w64.max())
    wsm /= wsm.sum()
    b = 1.0 / (1.0 + np.exp(-float(beta[0])))
    bwsm = np.tile((b / (1.0 - b) * wsm)[None, :], (128, 1)).astype(np.float32)
    ident = np.eye(128, dtype=np.float32)
    wza = np.zeros(CA * WBLK, np.float32).astype(ml_dtypes.bfloat16)
    wzb = np.zeros(CB * WBLK, np.float32).astype(ml_dtypes.bfloat16)

    in_maps = []
    for i in range(B):
        xp = np.zeros((NPAD, D), np.float32)
        xp[:N] = x[i]
        dtp = np.zeros(NPAD, np.float32)
        dtp[:N] = delta_times[i]
        vfp = np.zeros(NPAD, np.float32)
        vfp[:N] = valid_mask[i].astype(np.float32)
        dvf = np.zeros((99, 136), np.float32)
        for bb in range(NB):
            dvf[bb, :] = dtp[bb * BLK:bb * BLK + 136]
            dvf[64 + bb, :] = vfp[bb * BLK:bb * BLK + 136]
        in_maps.append({
            "x": xp.astype(ml_dtypes.bfloat16),
            "dvf": dvf,
            "vfr": dvf[64:99].copy(),
            "bwsm": bwsm,
            "idf": ident,
            "wza": wza,
            "wzb": wzb,
        })
    return in_maps


def _execute(in_maps, trace=False, **kw):
    nc = _get_nc()
    return run_bass_kernel_spmd(nc, in_maps, core_ids=list(range(B)),
                                trace=trace, **kw)


def kernel(x, delta_times, valid_mask, w, beta):
    in_maps = _make_in_maps(x, delta_times, valid_mask, w, beta)
    kr = _execute(in_maps, trace=False)
    outs = [kr.results[i]["out"][:N].astype(np.float32) for i in range(B)]
    return np.stack(outs, axis=0)


# revision 30
# speedup vs baseline: 1.5131x; 1.0105x over previous
"""AdaptiveTokenMixer Trainium2 kernel (8 NeuronCores, pure data parallel).

Per-core algorithm (one batch element per core), pipelined over 2 chunks
(18+17 position-blocks of BLK=120 outputs) mapped to the two HWDGE rings:
  1. alpha stage: delta_times/valid_mask host-packed into one [70, 136]
     row tensor; two PE transposes per tap produce both windows; t1 =
     BIG - dt fused into the scalar-engine PSUM evictions; cv/lg computed
     per-strip under the transpose stream; masked temporal-decay softmax
     over K=8 offsets (elementwise split across vector/gpsimd), blended as
     au = (e + s*c)*cv (scale-invariant rewrite avoids the reciprocal);
     alpha finalized per chunk -> af bf16.
  2. W stage (per chunk): af chunk written to a DRAM scratch with a SKEWED
     access pattern (banded W^T[m, k] = alpha[n0+m, k-m], m-major 128x128
     tiles over a zeros-initialized buffer); loaded back with an
     XBAR-transposing DMA into W[k, m] orientation. The skew and its XBAR
     load MUST be issued on opposite HWDGE rings (SP=sync / Act=scalar):
     a same-ring consumer's semaphore wait is elided under the ring-FIFO
     assumption, but the XBAR read races the skew's multi-engine
     descriptor drain (observed nondeterministic corruption).
  3. Mix (per block): out[m, :] = sum_k W[k, m] * x[n0+k, :] -- one
     128x128 @ 128x256 bf16 matmul per block (PSUM f32), two blocks per
     PSUM bank.
  4. Evict PSUM -> SBUF bf16 (paired, alternating vector/scalar), 5
     group-stores overlapping later matmuls.

Self-contained: hardcodes shapes for B=8, N=4096, d=256, K=8.
"""
import numpy as np
import ml_dtypes

import concourse.bass as bass
import concourse.bacc as bacc
import concourse.mybir as mybir
from concourse import tile
from concourse.bass_utils import run_bass_kernel_spmd

B, N, D, K = 8, 4096, 256, 8
BLK = 120                      # output positions per block
NB = (N + BLK - 1) // BLK      # 35 blocks -> covers 4200 positions
NOUT = NB * BLK                # 4200 rows in padded device output
NPAD = 4224                    # padded input length (>= 34*120 + 136)
KW = 128                       # k-window (contraction) per block
WBLK = KW * KW                 # W scratch elements per block
F = K * NB                     # alpha free size (b-major, p-minor)
BIG = 1024.0
CA = 18                        # chunk A blocks (skew on SP, load on Act)
CB = NB - CA                   # chunk B blocks (skew on Act, load on SP)
G = 5                          # store groups
GB = NB // G                   # blocks per store group (7)

_CACHE = {}


def _build():
    nc = bacc.Bacc("TRN2", target_bir_lowering=False, debug=False,
                   num_devices=B)
    f32 = mybir.dt.float32
    bf16 = mybir.dt.bfloat16

    x_t = nc.dram_tensor("x", [NPAD, D], bf16, kind="ExternalInput")
    dvf_t = nc.dram_tensor("dvf", [99, 136], f32, kind="ExternalInput")
    vfr_t = nc.dram_tensor("vfr", [35, 136], f32, kind="ExternalInput")
    bwsm_t = nc.dram_tensor("bwsm", [128, K], f32, kind="ExternalInput")
    idf_t = nc.dram_tensor("idf", [128, 128], f32, kind="ExternalInput")
    wza_t = nc.dram_tensor("wza", [CA * WBLK], bf16, kind="ExternalInput")
    wzb_t = nc.dram_tensor("wzb", [CB * WBLK], bf16, kind="ExternalInput")
    out_t = nc.dram_tensor("out", [NOUT, D], bf16, kind="ExternalOutput")

    def pb(t):  # [128,(b,p)] view -> [128, b, p] (p innermost, for reduces)
        return bass.AP(t.tensor, t.offset, [t.ap[0], [K, NB], [1, K]])

    def strip(t, p):  # tap-p strip [128, NB] (stride K)
        return bass.AP(t.tensor, t.offset + p, [t.ap[0], [K, NB]])

    def exp_nb_h(a, b0, nb):  # block-range slice of an exp_nb broadcast
        return bass.AP(a.tensor, a.offset + b0 * a.ap[1][0],
                       [a.ap[0], [a.ap[1][0], nb], [0, K]])

    def exp_k_h(a, nb):       # block-range slice of an exp_k broadcast
        return bass.AP(a.tensor, a.offset, [a.ap[0], [0, nb], [1, K]])

    NB1 = 24                  # block split for vector/gpsimd halves
    NB2 = NB - NB1
    F1 = NB1 * K

    def tt2(out, a, b2, op, b2h=None, ah=None):
        # tensor_tensor split across vector/gpsimd at block boundary NB1.
        if b2h is None:
            b2h = (b2[:, :F1], b2[:, F1:])
        if ah is None:
            ah = (a[:, :F1], a[:, F1:])
        nc.vector.tensor_tensor(out[:, :F1], ah[0], b2h[0], op)
        nc.gpsimd.tensor_tensor(out[:, F1:], ah[1], b2h[1], op)

    with tile.TileContext(nc) as tc:
        with tc.tile_pool(name="alph", bufs=1) as apool, \
             tc.tile_pool(name="outg", bufs=3) as opool, \
             tc.tile_pool(name="big", bufs=1) as bpool, \
             tc.tile_pool(name="psA", bufs=2, space="PSUM") as psA, \
             tc.tile_pool(name="psB", bufs=3, space="PSUM") as psB:

            # ---- input / constant loads (sync = SP ring) ----
            dvf = apool.tile([99, 136], f32)
            nc.sync.dma_start(dvf[:], dvf_t.ap())
            ident_f = apool.tile([128, 128], f32)
            nc.sync.dma_start(ident_f[:], idf_t.ap())
            vfr = apool.tile([35, 136], f32)
            nc.sync.dma_start(vfr[:], vfr_t.ap())
            bwsm = apool.tile([128, K], f32)
            nc.sync.dma_start(bwsm[:], bwsm_t.ap())
            # x windows in two chunks: x_all[i, b, d] = x[b*120+i, d]
            x_all = bpool.tile([128, NB, D], bf16)
            for c0, cn in ((0, CA), (CA, CB)):
                nc.sync.dma_start(
                    x_all[:, c0:c0 + cn, :],
                    bass.AP(x_t, c0 * BLK * D,
                            [[D, 128], [BLK * D, cn], [1, D]]))

            # ---- window strips: two PE transposes per tap ----
            # dvf rows 0..34 = dt blocks, rows 64..98 = vf blocks.
            # t1 = BIG - dt fused into the scalar PSUM eviction.
            t1 = apool.tile([128, F], f32)
            vw = apool.tile([128, F], f32)
            cv = apool.tile([128, F], f32)
            lg = apool.tile([128, F], f32)
            for p in range(K):
                ptw = psA.tile([128, 70], f32, tag="win")
                nc.tensor.transpose(ptw[:, 0:NB], dvf[:NB, p:p + 128],
                                    ident_f[:NB, :NB])
                nc.tensor.transpose(ptw[:, NB:70], vfr[:NB, p:p + 128],
                                    ident_f[:NB, :NB])
                nc.scalar.activation(strip(t1, p), ptw[:, 0:NB],
                                     mybir.ActivationFunctionType.Copy,
                                     bias=BIG, scale=-1.0)
                nc.vector.tensor_copy(strip(vw, p), ptw[:, NB:70])
                # cv_p = vw_p * vw_0; lg_p = t1_p * cv_p (under the stream)
                nc.vector.tensor_tensor(strip(cv, p), strip(vw, p),
                                        strip(vw, 0), mybir.AluOpType.mult)
                nc.gpsimd.tensor_tensor(strip(lg, p), strip(t1, p),
                                        strip(cv, p), mybir.AluOpType.mult)

            # ---- alpha stage ----
            mx = apool.tile([128, NB], f32)
            nc.vector.tensor_reduce(mx[:], pb(lg), mybir.AxisListType.X,
                                    mybir.AluOpType.max)
            ei = apool.tile([128, F], f32)
            tt2(ei, lg, None, mybir.AluOpType.subtract,
                b2h=(exp_nb_h(mx[:, :], 0, NB1), exp_nb_h(mx[:, :], NB1, NB2)))
            e = apool.tile([128, F], f32)
            nc.scalar.activation(e[:], ei[:], mybir.ActivationFunctionType.Exp)
            s = apool.tile([128, NB], f32)
            nc.vector.tensor_reduce(s[:], pb(e), mybir.AxisListType.X,
                                    mybir.AluOpType.add)
            # scale-invariant blend: au = (e + s*c) * cv  (c = bwsm row)
            sc = apool.tile([128, F], f32)
            tt2(sc, None, None, mybir.AluOpType.mult,
                ah=(exp_nb_h(s[:, :], 0, NB1), exp_nb_h(s[:, :], NB1, NB2)),
                b2h=(exp_k_h(bwsm[:, :], NB1), exp_k_h(bwsm[:, :], NB2)))
            au = apool.tile([128, F], f32)
            tt2(au, sc, e, mybir.AluOpType.add)
            tt2(au, au, cv, mybir.AluOpType.mult)
            sa = apool.tile([128, NB], f32)
            nc.vector.tensor_reduce(sa[:], pb(au), mybir.AxisListType.X,
                                    mybir.AluOpType.add)
            nc.vector.tensor_scalar(sa[:], sa[:], 1e-8, None,
                                    mybir.AluOpType.max)
            r = apool.tile([128, NB], f32)
            nc.vector.reciprocal(r[:], sa[:])
            nc.vector.tensor_tensor(r[:], r[:], strip(vw, 0),
                                    mybir.AluOpType.mult)
            # finalize alpha per chunk so skews start early
            af = apool.tile([128, F], bf16)
            nc.vector.tensor_tensor(af[:, :CA * K], au[:, :CA * K],
                                    exp_nb_h(r[:, :], 0, CA),
                                    mybir.AluOpType.mult)
            nc.gpsimd.tensor_tensor(af[:, CA * K:], au[:, CA * K:],
                                    exp_nb_h(r[:, :], CA, CB),
                                    mybir.AluOpType.mult)

            # ---- skewed W writes: W^T[b][m, m+p] = af[m, p, b] ----
            # chunk A skew on SP ring, chunk B skew on Act ring
            nc.sync.dma_start(
                bass.AP(wza_t, 0, [[KW + 1, BLK], [WBLK, CA], [1, K]]),
                bass.AP(af.tensor, af.offset,
                        [af.ap[0], [K, CA], [1, K]])[:BLK, :, :])
            nc.sync.dma_start(
                bass.AP(wzb_t, 0, [[KW + 1, BLK], [WBLK, CB], [1, K]]),
                bass.AP(af.tensor, af.offset + CA * K,
                        [af.ap[0], [K, CB], [1, K]])[:BLK, :, :])

            # ---- XBAR-transposed loads (opposite ring from the skew) ----
            # NOTE: the XBAR is one shared unit -- concurrent DMA_TRANSPOSE
            # instructions from the two HWDGE rings corrupt each other.
            # All transposes must serialize on one ring (Act).
            w_all = bpool.tile([128, NB, KW], bf16)
            nc.scalar.dma_start(
                w_all[:, 0:CA, :],
                bass.AP(wza_t, 0, [[KW, CA * KW], [1, KW]]),
                transpose=True)
            nc.scalar.dma_start(
                w_all[:, CA:NB, :],
                bass.AP(wzb_t, 0, [[KW, CB * KW], [1, KW]]),
                transpose=True)

            # ---- per-block banded matmul; paired evict; group stores ----
            for g in range(G):
                out_g = opool.tile([128, GB, D], bf16, tag="og")
                for j in range(0, GB, 2):
                    b = g * GB + j
                    npair = min(2, GB - j)
                    pt = psB.tile([KW, 2 * D], f32, tag="mm")
                    for q in range(npair):
                        nc.tensor.matmul(pt[:, q * D:(q + 1) * D],
                                         w_all[:, b + q, :],
                                         x_all[:, b + q, :])
                    if (j // 2) % 2 == 1:
                        nc.scalar.copy(out_g[:BLK, j:j + npair, :],
                                       pt[:BLK, :npair * D])
                    else:
                        nc.vector.tensor_copy(out_g[:BLK, j:j + npair, :],
                                              pt[:BLK, :npair * D])
                nc.sync.dma_start(
                    bass.AP(out_t, g * GB * BLK * D,
                            [[D, BLK], [BLK * D, GB], [1, D]]),
                    out_g[:BLK, :, :])
    nc.compile()
    return nc


def _get_nc():
    if "nc" not in _CACHE:
        _CACHE["nc"] = _build()
    return _CACHE["nc"]


def _make_in_maps(x, delta_times, valid_mask, w, beta):
    w64 = w.astype(np.float64)
    wsm = np.exp(w64 - w64.max())
    wsm /= wsm.sum()
    b = 1.0 / (1.0 + np.exp(-float(beta[0])))
    bwsm = np.tile((b / (1.0 - b) * wsm)[None, :], (128, 1)).astype(np.float32)
    ident = np.eye(128, dtype=np.float32)
    wza = np.zeros(CA * WBLK, np.float32).astype(ml_dtypes.bfloat16)
    wzb = np.zeros(CB * WBLK, np.float32).astype(ml_dtypes.bfloat16)

    in_maps = []
    for i in range(B):
        xp = np.zeros((NPAD, D), np.float32)
        xp[:N] = x[i]
        dtp = np.zeros(NPAD, np.float32)
        dtp[:N] = delta_times[i]
        vfp = np.zeros(NPAD, np.float32)
        vfp[:N] = valid_mask[i].astype(np.float32)
        dvf = np.zeros((99, 136), np.float32)
        for bb in range(NB):
            dvf[bb, :] = dtp[bb * BLK:bb * BLK + 136]
            dvf[64 + bb, :] = vfp[bb * BLK:bb * BLK + 136]
        in_maps.append({
            "x": xp.astype(ml_dtypes.bfloat16),
            "dvf": dvf,
            "vfr": dvf[64:99].copy(),
            "bwsm": bwsm,
            "idf": ident,
            "wza": wza,
            "wzb": wzb,
        })
    return in_maps


def _execute(in_maps, trace=False, **kw):
    nc = _get_nc()
    return run_bass_kernel_spmd(nc, in_maps, core_ids=list(range(B)),
                                trace=trace, **kw)


def kernel(x, delta_times, valid_mask, w, beta):
    in_maps = _make_in_maps(x, delta_times, valid_mask, w, beta)
    kr = _execute(in_maps, trace=False)
    outs = [kr.results[i]["out"][:N].astype(np.float32) for i in range(B)]
    return np.stack(outs, axis=0)
